# revision 46
# baseline (speedup 1.0000x reference)
"""CTPNet Trainium2 kernel: 8-way data-parallel over the batch dim.

Net (per reference):
    h1 = relu(x @ W1.T + b1)            x:[B,G]  W1:[H1,G]   -> [B,512]
    h2 = relu(h1 @ W2.T + b2)           W2:[H2,H1]           -> [B,256]
    a  = relu(einsum(bh,pha->bpa) + b3) W3:[P,H2,A]          -> [B,P,A]
    out= einsum(bpa,pa->bp) + b4        W4:[P,A]             -> [B,P]

B,G,H1,H2,P,A = 8192, 20000, 512, 256, 128, 64

Strategy: shard x rows 8 ways (1024 rows/core), replicate weights.
All on-chip tensors live in the "transposed" layout (feature dim on
partitions, batch on the free axis) so every layer is a plain chain of
TensorE matmuls with K (contraction) on the partition dim:

    h1T[512,1024]  = W1T.T-tiles @ xT-tiles          (157 K-tiles over G)
    h2T[256,1024]  = W2T-tiles @ h1T
    aT  [8192,1024] = W3f-tiles @ h2T  (heads flattened to [H2, P*A])
    outT[128,1024] = W4bd-tiles @ aT   (block-diag W4 does the A-reduction)

Host pre-transposes weights/x so every DMA is contiguous; host
re-assembles out = concat(outT_c.T).  Biases are folded into the
PSUM->SBUF eviction (Relu with per-partition bias, split across
ScalarE/VectorE).

Default mode v10 (~336 us/core, fro rel err 1.79e-2 vs the f32
reference; v3 history at the bottom) = v3 + two structural changes:

(1) fp8 split-K L1 (_build_v9, NP8=20 DoubleRow pairs): k-tiles
    0..39 of the G contraction run as e4m3 DoubleRow matmuls (lhsT
    [128,2,128] / rhs [128,2,BW] via AP rearrange, 2 k-tiles per
    512-cycle pass = 2x rate), the remaining 117 tiles stay bf16 and
    accumulate into the same PSUM banks.  Scales x*2^-1, W1*2^6
    (exact pow2, both fp8 AND bf16 slices -> PSUM = 2^5*z1); the
    descale is free: host scales b1*2^5 (h1a = 2^5*h1) and W2*2^-5
    (z2 exact; downstream untouched).  Error scales ~sqrt(NP8):
    e4m3 RTN is ~4.5% rms/operand -> 3.44e-2 at full-K (why plain
    fp8 failed before); NP8=20 measures 1.79e-2 on HW (=CPU-sim
    prediction to 3 digits; PE upcasts e4m3->e6m3 exactly).  NP8=22
    -> 1.875e-2 also passes but leaves only 6% gate margin.
(2) L4 hybrid off the PE (mode v10, NPE=30): W3 host-permuted to
    [H2, A, P] so each L3 chunk-matmul emits psum[protein, b] for
    ONE a (same 131K PE cycles, same moving operands, b3[:,a]/w4[:,a]
    per-partition).  The W4 dot over a then leaves the PE: 34 a's
    accumulate via vector.scalar_tensor_tensor fp16 chains
    (acc = at*w4[:,a] + acc, two independent chains/b-chunk, b4
    seeds chain 0, pre-merged before the PE drains), 30 a's stay on
    the PE as diag(W4[:,a]) matmuls into 2 PSUM banks (pst1/pst3),
    evictions 3:1 Scalar:Vector.  Old block-diag L4 cost 65.5K PE
    cycles for 0.13 GFLOP; the hybrid costs 30.7K PE cycles + ~50us
    of otherwise-idle V/S.  NPE sweep: 22->347us 28->341 30->336.7
    36->340 44->340 64(v9, all-PE)->348; all-DVE (NPE=0, f32 acc)
    ->397us: scalar_tensor_tensor is a 2-port DVE op, 741ns/512cols
    REGARDLESS of dtype (2X_1PORT does not apply), and Scalar
    activation evictions measure 687ns, so S/V saturate if the PE
    share drops below ~30.  TensorScalarPtr (tensor_scalar with AP
    scalar, and scalar_tensor_tensor) is INVALID on GpSimd: walrus
    "Instruction engine check failed (Pool)" -- GpSimd cannot help.
    bf16 acc chains add 7e-3 error (too much); fp16 adds <1e-3.

Measured budget at 336us: PE active 318us (745.6K cycles: L1 561K,
L2 8K, L3 131K, L4-diag 31K; gapless but for ~3.5us of ramp gaps),
~12.5us NEFF/DMA startup before the first matmul, ~6us tail
(po-merge + out DMA + drain), throttle ~9us from the cold start.
W4d diag + W3/W4 prefetch DMAs issue at PREFETCH_AT (mid-L1):
putting the 1MB w4d DMA at t=0 cost ~2us of ramp.  fp8 ramp
[1,1,2,3,4,4]+[4...] with 3 bufs of 4-pair groups; the finer ramps
[1,1,2,2,3,3...] and [1,1,1,2,3,4...] both STALLED the PE ~3-6us
around t=18-22us (and re-throttled the clock) -- do not re-tune
the ramp by +-1 groups, measured deltas are +-2.5us run noise.

v3 history (~379 us/core, rel err 4.4e-3): bf16 compute, 8-k-tile
DMA groups, single PSUM pool with per-bank tag chaining across
L1->L2->L3/L4, W2/W3/W4 fully prefetched into SBUF during L1,
consumer-ordered evictions, software-pipelined L4, split tail
eviction+DMA; PE gapless at 2.37 GHz for ~358 of the 379.  Dead
ends kept as modes: HAM warm-up dummies (v4), split first-tile DMA
(v5/v7 -- locks the clock at 1.98 GHz all run!), 16-tile groups
(v6), tail region-split (v8).  CTP_MODE=f32r: f32r end-to-end
(~494 us, DMA-bound).
"""

import os
import numpy as np

B, G, H1, H2, P, A = 8192, 20000, 512, 256, 128, 64
NCORES = 8
BC = B // NCORES            # 1024 batch rows per core
NBC = 2                     # b-chunks per core
BW = BC // NBC              # 512 (one PSUM bank / max fp32 moving free dim)
KT = 128
NKP = 157                   # K-tiles over G (156 full + one K=32 tail)
GP = NKP * KT               # 20096 (zero-padded from 20000)
KG = 4                      # max k-tiles per DMA group
# ramp-up schedule: small first chunks so the PE starts ~10us sooner
_sizes = [1, 1, 2] + [KG] * ((NKP - 4) // KG) + (
    [NKP - 4 - KG * ((NKP - 4) // KG)] if (NKP - 4) % KG else []
)
KGROUPS = []
_k = 0
for _s in _sizes:
    KGROUPS.append((_k, _s))
    _k += _s
assert _k == NKP
PA = P * A                  # 8192 flattened head outputs
NM3 = PA // 128             # 64 pa-chunks
NM1 = H1 // 128             # 4
NK2 = H1 // 128             # 4
NM2 = H2 // 128             # 2
NK3 = H2 // 128             # 2

_CACHE = {}


def _build(mode: str):
    """Build + compile the per-core Bass graph. mode: 'f32r' | 'f32' | 'bf16'."""
    import concourse.bacc as bacc
    import concourse.tile as tile
    import concourse.mybir as mybir
    from contextlib import ExitStack

    FP = mybir.dt.float32
    # ST: storage dtype of streamed/intermediate SBUF tiles (and big DRAM ins)
    ST = mybir.dt.bfloat16 if mode.startswith("bf16") else (
        mybir.dt.float32r if mode == "f32r" else mybir.dt.float32
    )
    DEEP = 6 if mode in ("bf16x", "bf16y", "bf16w") else 4
    PS3B = 5 if mode == "bf16y" else 4
    WARM = mode == "bf16w"  # HAM keep-warm dummy matmuls
    L4LAG = 2 if mode == "bf16w" else 1
    CHAIN = mode == "bf16t"  # single PSUM pool, per-bank tag chaining
    if mode in ("bf16v", "bf16u", "bf16t", "bf16s"):
        # fewer DMA groups: halves the per-group first-matmul sem-wait tax
        KGv = 16 if mode == "bf16s" else 8
        if mode == "bf16s":
            # halve the group count again: fewer first-matmul sem-wait taxes
            sizes_v = [1, 1, 2, 4, 8] + [16] * 8 + [13]
        elif mode == "bf16u":
            # gentler ramp + deeper buffers: kill the early-L1 DMA-pacing
            # stalls seen at t=17-31us in the bf16v trace
            sizes_v = [1, 1, 2, 4, 4] + [8] * 18 + [1]
        else:
            sizes_v = [1, 1, 2, 4] + [8] * 18 + [5]  # bf16v/bf16t
        kgroups = []
        kk0 = 0
        for sz in sizes_v:
            kgroups.append((kk0, sz))
            kk0 += sz
        assert kk0 == NKP
        xbufs = 4 if mode == "bf16u" else 3
        MIB = 4   # heads-chunks per W3/W4 load in the tail
    else:
        KGv = KG
        kgroups = KGROUPS
        xbufs = 6 if mode.startswith("bf16") else 4
        MIB = 1

    def mc(ap):
        return ap

    nc = bacc.Bacc(
        "TRN2", target_bir_lowering=False, debug=False, num_devices=NCORES
    )

    # k-tile-interleaved layouts: col block ki holds k-tile ki
    # xTi[p, ki*BC + j]  = x_core[j, ki*128 + p]   (zero-padded past G)
    # w1ti[p, ki*H1 + h] = W1[h, ki*128 + p]
    xT_d = nc.dram_tensor("xTi", [128, NKP * BC], ST, kind="ExternalInput")
    w1t_d = nc.dram_tensor("w1ti", [128, NKP * H1], ST, kind="ExternalInput")
    b1r_d = nc.dram_tensor("b1r", [128, NM1], FP, kind="ExternalInput")
    w2t_d = nc.dram_tensor("w2t", [H1, H2], ST, kind="ExternalInput")
    b2r_d = nc.dram_tensor("b2r", [128, NM2], FP, kind="ExternalInput")
    w3f_d = nc.dram_tensor("w3f", [H2, PA], ST, kind="ExternalInput")
    b3r_d = nc.dram_tensor("b3r", [128, NM3], FP, kind="ExternalInput")
    w4i_d = nc.dram_tensor("w4i", [128, NM3 * 128], ST, kind="ExternalInput")
    b4r_d = nc.dram_tensor("b4r", [128, 1], FP, kind="ExternalInput")
    out_d = nc.dram_tensor("out", [P, BC], FP, kind="ExternalOutput")

    Relu = mybir.ActivationFunctionType.Relu

    with tile.TileContext(nc) as tc:
        with (
            tc.tile_pool(name="const", bufs=1) as const,
            tc.tile_pool(name="h1", bufs=1) as h1pool,
            tc.tile_pool(name="h2", bufs=1) as h2pool,
            tc.tile_pool(name="osb", bufs=1) as opool,
            tc.tile_pool(name="xp", bufs=xbufs) as xpool,
            tc.tile_pool(name="w1p", bufs=xbufs) as w1pool,
        ):
            b1t = const.tile([128, NM1], FP)
            nc.scalar.dma_start(b1t[:], b1r_d[:])
            b2t = const.tile([128, NM2], FP)
            nc.scalar.dma_start(b2t[:], b2r_d[:])
            b3t = const.tile([128, NM3], FP)
            nc.scalar.dma_start(b3t[:], b3r_d[:])
            b4t = const.tile([128, 1], FP)
            nc.scalar.dma_start(b4t[:], b4r_d[:])

            # h1T as [128, m(4) x b(1024)]; col m*BC + j holds h1[m*128+p, j]
            h1a = h1pool.tile([128, NM1 * BC], ST)
            h2a = h2pool.tile([128, NM2 * BC], ST)
            outsb = opool.tile([128, BC], FP)
            warmt = None
            if WARM:
                warmt = const.tile([128, 64], ST)
                nc.vector.memset(warmt[:], 0.0)

            def warm_mms(ps_ap, n):
                # tiny matmuls on zeros: keep the PE-HAM activity window
                # busy through a stretch where the real stream would idle
                # (idle > ~3.4us rethrottles the PE clock to 1.2 GHz).
                # The consumer's start=True overwrites the garbage.
                for _ in range(n):
                    nc.tensor.matmul(
                        ps_ap[0:64, 0:64], warmt[:, 0:64], warmt[:, 0:64],
                        start=True, stop=True,
                    )

            # ---- L1: h1T = relu(W1T.T @ xT + b1), K over G ----
            pse = ExitStack()
            if True:
                ps1 = pse.enter_context(
                    tc.tile_pool(name="ps1", bufs=1, space="PSUM")
                )
                pst = [
                    ps1.tile([128, BW], FP, name=f"pst{i}", tag=f"pst{i}")
                    for i in range(NM1 * NBC)
                ]
                if WARM:
                    warm_mms(pst[0], 64)
                for (k0, gsz) in kgroups:
                    wt = w1pool.tile([128, KGv * H1], ST)
                    nc.sync.dma_start(
                        wt[:, : gsz * H1], w1t_d[:, k0 * H1 : (k0 + gsz) * H1]
                    )
                    xt = xpool.tile([128, KGv * BC], ST)
                    nc.sync.dma_start(
                        xt[:, : gsz * BC], xT_d[:, k0 * BC : (k0 + gsz) * BC]
                    )
                    # k-contiguous per PSUM bank: bank switches every gsz MMs
                    # (not every MM) to avoid psum-queue depth-cycling stalls
                    for m in range(NM1):
                        for b in range(NBC):
                            for kk in range(gsz):
                                ki = k0 + kk
                                nc.tensor.matmul(
                                    pst[m * NBC + b][:, :],
                                    mc(wt[:, kk * H1 + m * 128 : kk * H1 + (m + 1) * 128]),
                                    mc(xt[:, kk * BC + b * BW : kk * BC + (b + 1) * BW]),
                                    start=(ki == 0),
                                    stop=(ki == NKP - 1),
                                )
                for m in range(NM1):
                    for b in range(NBC):
                        c0 = m * BC + b * BW
                        if (m * NBC + b) % 2 == 0:
                            nc.scalar.activation(
                                h1a[:, c0 : c0 + BW],
                                pst[m * NBC + b][:, :],
                                Relu,
                                bias=b1t[:, m : m + 1],
                            )
                        else:
                            nc.vector.tensor_scalar(
                                h1a[:, c0 : c0 + BW],
                                pst[m * NBC + b][:, :],
                                b1t[:, m : m + 1],
                                0.0,
                                op0=mybir.AluOpType.add,
                                op1=mybir.AluOpType.max,
                            )

            # ---- L2: h2T = relu(W2T.T @ h1T + b2), K over H1 ----
            # CHAIN: L2 accumulators reuse L1's banks per-tag, so allocation
            # waits only for that bank's eviction -- not the whole pool close
            with tc.tile_pool(name="w2p", bufs=NK2) as w2pool:
                if CHAIN:
                    ps2 = ps1
                else:
                    pse.close()
                    pse = ExitStack()
                    ps2 = pse.enter_context(
                        tc.tile_pool(name="ps2", bufs=1, space="PSUM")
                    )
                pst2 = [
                    ps2.tile(
                        [128, BW], FP, name=f"pst2_{i}",
                        tag=(f"pst{i}" if CHAIN else f"pst2_{i}"),
                    )
                    for i in range(NM2 * NBC)
                ]
                if WARM:
                    warm_mms(pst2[0], 24)
                for ki in range(NK2):
                    w2t = w2pool.tile([128, H2], ST)
                    nc.scalar.dma_start(w2t[:], w2t_d[ki * 128 : (ki + 1) * 128, :])
                    for m in range(NM2):
                        for b in range(NBC):
                            nc.tensor.matmul(
                                pst2[m * NBC + b][:, :],
                                mc(w2t[:, m * 128 : (m + 1) * 128]),
                                mc(h1a[:, ki * BC + b * BW : ki * BC + b * BW + BW]),
                                start=(ki == 0),
                                stop=(ki == NK2 - 1),
                            )
                for m in range(NM2):
                    for b in range(NBC):
                        c0 = m * BC + b * BW
                        if (m * NBC + b) % 2 == 0:
                            nc.scalar.activation(
                                h2a[:, c0 : c0 + BW],
                                pst2[m * NBC + b][:, :],
                                Relu,
                                bias=b2t[:, m : m + 1],
                            )
                        else:
                            nc.vector.tensor_scalar(
                                h2a[:, c0 : c0 + BW],
                                pst2[m * NBC + b][:, :],
                                b2t[:, m : m + 1],
                                0.0,
                                op0=mybir.AluOpType.add,
                                op1=mybir.AluOpType.max,
                            )

            # ---- L3+L4: aT chunks then block-diag W4 reduction ----
            with (
                tc.tile_pool(name="w3p", bufs=DEEP) as w3pool,
                tc.tile_pool(name="w4p", bufs=DEEP) as w4pool,
                tc.tile_pool(name="ap", bufs=DEEP) as apool,
            ):
                if CHAIN:
                    ps4 = ps1
                    ps3pool = ps1
                else:
                    pse.close()
                    pse = ExitStack()
                    ps4 = pse.enter_context(
                        tc.tile_pool(name="ps4", bufs=1, space="PSUM")
                    )
                    ps3pool = pse.enter_context(
                        tc.tile_pool(name="ps3", bufs=PS3B, space="PSUM")
                    )
                po = [
                    ps4.tile(
                        [128, BW], FP, name=f"po{i}",
                        tag=(f"pst{4 + i}" if CHAIN else f"po{i}"),
                    )
                    for i in range(NBC)
                ]
                if WARM:
                    warm3 = ps3pool.tile([128, BW], FP, name="warm3", tag="ps3")
                    warm_mms(warm3, 16)
                # software-pipelined: L4 accumulation for step mi-1 is emitted
                # between step mi's L3 matmuls so the PSUM->SBUF eviction
                # latency never blocks the PE stream.
                pend = []  # (mi, b, w4t, at) awaiting their L4 matmul

                def flush_l4(upto=None):
                    keep = []
                    for (pmi, pb, pw4t, pat) in pend:
                        if upto is not None and pmi > upto:
                            keep.append((pmi, pb, pw4t, pat))
                            continue
                        nc.tensor.matmul(
                            po[pb][:, :],
                            mc(pw4t),
                            mc(pat[:, :]),
                            start=(pmi == 0),
                            stop=(pmi == NM3 - 1),
                        )
                    pend[:] = keep

                for mi in range(NM3):
                    ml = mi % MIB
                    if ml == 0:
                        w3t = w3pool.tile([128, MIB * H2], ST)
                        for k in range(NK3):
                            nc.sync.dma_start(
                                w3t[:, k * MIB * 128 : (k * MIB + MIB) * 128],
                                w3f_d[k * 128 : (k + 1) * 128,
                                      mi * 128 : (mi + MIB) * 128],
                            )
                        w4t = w4pool.tile([128, MIB * 128], ST)
                        nc.sync.dma_start(
                            w4t[:], w4i_d[:, mi * 128 : (mi + MIB) * 128]
                        )
                    mypend = []
                    for b in range(NBC):
                        _i3 = mi * NBC + b
                        ps3 = ps3pool.tile(
                            [128, BW], FP, name=f"ps3_{_i3}",
                            tag=(f"pst{_i3 % 4}" if CHAIN else "ps3"),
                        )
                        for k in range(NK3):
                            nc.tensor.matmul(
                                ps3[:, :],
                                mc(w3t[:, (k * MIB + ml) * 128 : (k * MIB + ml + 1) * 128]),
                                mc(h2a[:, k * BC + b * BW : k * BC + b * BW + BW]),
                                start=(k == 0),
                                stop=(k == NK3 - 1),
                            )
                        if b == NBC - 1:
                            # L4 for step mi-L4LAG: gives the eviction chain
                            # L4LAG steps of slack before the PE needs `at`
                            flush_l4(upto=mi - L4LAG)
                        at = apool.tile([128, BW], ST)
                        if (mi * NBC + b) % 5 < 3:
                            nc.scalar.activation(
                                at[:, :], ps3[:, :], Relu, bias=b3t[:, mi : mi + 1]
                            )
                        else:
                            # relu(x + b3) on VectorE: (x add b3) max 0
                            nc.vector.tensor_scalar(
                                at[:, :],
                                ps3[:, :],
                                b3t[:, mi : mi + 1],
                                0.0,
                                op0=mybir.AluOpType.add,
                                op1=mybir.AluOpType.max,
                            )
                        mypend.append((mi, b, w4t[:, ml * 128 : (ml + 1) * 128], at))
                    pend.extend(mypend)
                flush_l4()
                for b in range(NBC):
                    nc.vector.tensor_scalar_add(
                        outsb[:, b * BW : (b + 1) * BW], po[b][:, :], b4t[:, 0:1]
                    )
            pse.close()
            nc.sync.dma_start(out_d[:, :], outsb[:, :])

    nc.compile()
    return nc


def _build_v3(mode: str):
    """Scheduling-optimized bf16 build.

    vs bf16u: (1) single PSUM pool with per-bank tag chaining across
    L1->L2->L3/L4 so phase N+1's first matmul waits only on one bank's
    eviction, not a pool close; (2) W2/W3/W4 fully prefetched into SBUF
    during L1 (L3/L4 phase does zero DMA); (3) L1 evictions spread over
    Scalar/Vector/GpSimd, L2 evictions ordered b=0-first to unblock L3;
    (4) smoother DMA ramp; (5) split final eviction+DMA per b-chunk so
    the out DMA trigger latency overlaps the last evictions.
    """
    import concourse.bacc as bacc
    import concourse.tile as tile
    import concourse.mybir as mybir

    FP = mybir.dt.float32
    ST = mybir.dt.bfloat16
    if mode == "v6":
        KGv = 16
        sizes_v = [1, 1, 2, 3, 4, 5, 6] + [16] * 8 + [7]
        xbufs_n = 2
    else:
        KGv = 8
        sizes_v = [1, 1, 2, 3, 4, 5, 6] + [8] * 16 + [7]
        xbufs_n = 4
    assert sum(sizes_v) == NKP
    kgroups = []
    _k0 = 0
    for _s in sizes_v:
        kgroups.append((_k0, _s))
        _k0 += _s
    PREFETCH_AT = 18 if mode != "v6" else 11  # W3/W4 prefetch DMA issue point
    xbufs = xbufs_n
    V4 = mode in ("v4", "v5")
    G0S = mode in ("v4", "v5", "v7")
    TAILS = mode in ("v4", "v5", "v8")
    # v4's HAM warm-up experiment regressed: dummy matmuls are themselves
    # cold-clock-limited (53ns each) and delay the real stream, while the
    # cold-clock real start is well-matched to the slow early DMA ramp.
    NWARM = 130 if mode == "v4" else 0
    Relu = mybir.ActivationFunctionType.Relu
    Ident = mybir.ActivationFunctionType.Identity

    nc = bacc.Bacc(
        "TRN2", target_bir_lowering=False, debug=False, num_devices=NCORES
    )

    xT_d = nc.dram_tensor("xTi", [128, NKP * BC], ST, kind="ExternalInput")
    w1t_d = nc.dram_tensor("w1ti", [128, NKP * H1], ST, kind="ExternalInput")
    b1r_d = nc.dram_tensor("b1r", [128, NM1], FP, kind="ExternalInput")
    w2i_d = nc.dram_tensor("w2i", [128, NK2 * H2], ST, kind="ExternalInput")
    b2r_d = nc.dram_tensor("b2r", [128, NM2], FP, kind="ExternalInput")
    w3f_d = nc.dram_tensor("w3f", [H2, PA], ST, kind="ExternalInput")
    b3r_d = nc.dram_tensor("b3r", [128, NM3], FP, kind="ExternalInput")
    w4i_d = nc.dram_tensor("w4i", [128, NM3 * 128], ST, kind="ExternalInput")
    b4r_d = nc.dram_tensor("b4r", [128, 1], FP, kind="ExternalInput")
    out_d = nc.dram_tensor("out", [P, BC], FP, kind="ExternalOutput")

    with tile.TileContext(nc) as tc:
        with (
            tc.tile_pool(name="const", bufs=1) as const,
            tc.tile_pool(name="wpre", bufs=1) as wpre,
            tc.tile_pool(name="h1", bufs=1) as h1pool,
            tc.tile_pool(name="h2", bufs=1) as h2pool,
            tc.tile_pool(name="osb", bufs=1) as opool,
            tc.tile_pool(name="xp", bufs=xbufs) as xpool,
            tc.tile_pool(name="w1p", bufs=xbufs) as w1pool,
            tc.tile_pool(name="ap", bufs=12) as apool,
            tc.tile_pool(name="ps", bufs=1, space="PSUM") as ps,
        ):
            b1t = const.tile([128, NM1], FP)
            nc.scalar.dma_start(b1t[:], b1r_d[:])
            b2t = const.tile([128, NM2], FP)
            nc.scalar.dma_start(b2t[:], b2r_d[:])
            b3t = const.tile([128, NM3], FP)
            nc.scalar.dma_start(b3t[:], b3r_d[:])
            b4t = const.tile([128, 1], FP)
            nc.scalar.dma_start(b4t[:], b4r_d[:])
            # W2 is tiny and needed right after L1: load it up front on the
            # gpsimd queue (idle at start, doesn't contend with the x ramp).
            w2a = wpre.tile([128, NK2 * H2], ST)
            nc.gpsimd.dma_start(w2a[:], w2i_d[:])
            # W3/W4 prefetch buffers; DMAs issue mid-L1 (see loop below) so
            # the transfers slot into the x-stream's spare bandwidth.
            w3a = wpre.tile([128, PA], ST)
            w3b = wpre.tile([128, PA], ST)
            w4a = wpre.tile([128, NM3 * 128], ST)

            h1a = h1pool.tile([128, NM1 * BC], ST)
            h2a = h2pool.tile([128, NM2 * BC], ST)
            outsb = opool.tile([128, BC], FP)

            pst = [
                ps.tile([128, BW], FP, name=f"pst{i}", tag=f"pst{i}")
                for i in range(NM1 * NBC)
            ]

            if NWARM:
                # HAM warm-up: zero-dependency dummy matmuls run during the
                # startup DMA window so the PE clock is at full rate (and the
                # pipeline hot) when the first real k-tile lands.  Consumed
                # by nothing; pst[0]'s real k=0 matmul start=True overwrites.
                warmt = const.tile([128, 64], ST)
                nc.vector.memset(warmt[:], 0.0)
                for _ in range(NWARM):
                    nc.tensor.matmul(
                        pst[0][0:64, 0:64], warmt[:, 0:64], warmt[:, 0:64],
                        start=True, stop=True,
                    )

            # ---- L1: h1T = relu(W1T.T @ xT + b1), K over G ----
            for gi, (k0, gsz) in enumerate(kgroups):
                if gi == PREFETCH_AT:
                    nc.sync.dma_start(w3a[:], w3f_d[0:128, :])
                    nc.sync.dma_start(w3b[:], w3f_d[128:256, :])
                    nc.sync.dma_start(w4a[:], w4i_d[:])
                wt = w1pool.tile([128, KGv * H1], ST)
                xt = xpool.tile([128, KGv * BC], ST)
                if gi == 0 and G0S:
                    # split the first tile's transfers so matmul (m0,b0)
                    # waits on 160KB, not 384KB
                    for m in range(NM1):
                        nc.sync.dma_start(
                            wt[:, m * 128 : (m + 1) * 128],
                            w1t_d[:, m * 128 : (m + 1) * 128],
                        )
                    for b in range(NBC):
                        nc.sync.dma_start(
                            xt[:, b * BW : (b + 1) * BW],
                            xT_d[:, b * BW : (b + 1) * BW],
                        )
                else:
                    nc.sync.dma_start(
                        wt[:, : gsz * H1], w1t_d[:, k0 * H1 : (k0 + gsz) * H1]
                    )
                    nc.sync.dma_start(
                        xt[:, : gsz * BC], xT_d[:, k0 * BC : (k0 + gsz) * BC]
                    )
                for m in range(NM1):
                    for b in range(NBC):
                        for kk in range(gsz):
                            ki = k0 + kk
                            nc.tensor.matmul(
                                pst[m * NBC + b][:, :],
                                wt[:, kk * H1 + m * 128 : kk * H1 + (m + 1) * 128],
                                xt[:, kk * BC + b * BW : kk * BC + (b + 1) * BW],
                                start=(ki == 0),
                                stop=(ki == NKP - 1),
                            )
            # L1 evictions: m-major so bank m*2+b frees in the order L2
            # consumes h1a m-blocks (GpSimd cannot read PSUM, so S/V only).
            for m in range(NM1):
                for b in range(NBC):
                    i = m * NBC + b
                    c0 = m * BC + b * BW
                    if i % 2 == 0:
                        nc.scalar.activation(
                            h1a[:, c0 : c0 + BW], pst[i][:, :], Relu,
                            bias=b1t[:, m : m + 1],
                        )
                    else:
                        nc.vector.tensor_scalar(
                            h1a[:, c0 : c0 + BW], pst[i][:, :],
                            b1t[:, m : m + 1], 0.0,
                            op0=mybir.AluOpType.add, op1=mybir.AluOpType.max,
                        )

            # ---- L2: h2T = relu(W2T.T @ h1T + b2), K over H1 ----
            pst2 = [
                ps.tile([128, BW], FP, name=f"pst2_{i}", tag=f"pst{i}")
                for i in range(NM2 * NBC)
            ]
            for ki in range(NK2):
                for m in range(NM2):
                    for b in range(NBC):
                        nc.tensor.matmul(
                            pst2[m * NBC + b][:, :],
                            w2a[:, ki * H2 + m * 128 : ki * H2 + (m + 1) * 128],
                            h1a[:, ki * BC + b * BW : ki * BC + b * BW + BW],
                            start=(ki == 0),
                            stop=(ki == NK2 - 1),
                        )
            # L2 evictions b=0-first (L3's first k-pair reads both m-blocks
            # of b=0) and on separate engines so they land together.
            for b in range(NBC):
                for m in range(NM2):
                    c0 = m * BC + b * BW
                    if m % 2 == 0:
                        nc.scalar.activation(
                            h2a[:, c0 : c0 + BW], pst2[m * NBC + b][:, :], Relu,
                            bias=b2t[:, m : m + 1],
                        )
                    else:
                        nc.vector.tensor_scalar(
                            h2a[:, c0 : c0 + BW], pst2[m * NBC + b][:, :],
                            b2t[:, m : m + 1], 0.0,
                            op0=mybir.AluOpType.add, op1=mybir.AluOpType.max,
                        )

            # ---- L3+L4: aT chunks then block-diag W4 reduction ----
            po = [
                ps.tile([128, BW], FP, name=f"po{i}", tag=f"pst{4 + i}")
                for i in range(NBC)
            ]
            # ps3 rotation tags: banks that free earliest after L1/L2.
            rot = ["pst6", "pst7", "pst2", "pst0"]
            w3ab = [w3a, w3b]
            pend = []  # (mi, b, w4_ap, at) awaiting their L4 matmul

            def flush_l4(upto=None):
                keep = []
                for (pmi, pb, pw4, pat) in pend:
                    if upto is not None and pmi > upto:
                        keep.append((pmi, pb, pw4, pat))
                        continue
                    nc.tensor.matmul(
                        po[pb][:, :], pw4, pat[:, :],
                        start=(pmi == 0), stop=(pmi == NM3 - 1),
                    )
                pend[:] = keep

            HWB = BW // 2
            for mi in range(NM3):
                for b in range(NBC):
                    i3 = mi * NBC + b
                    ps3 = ps.tile(
                        [128, BW], FP, name=f"ps3_{i3}", tag=rot[i3 % 4]
                    )
                    for k in range(NK3):
                        nc.tensor.matmul(
                            ps3[:, :],
                            w3ab[k][:, mi * 128 : (mi + 1) * 128],
                            h2a[:, k * BC + b * BW : k * BC + b * BW + BW],
                            start=(k == 0),
                            stop=(k == NK3 - 1),
                        )
                    if mi == NM3 - 1 and TAILS:
                        # tail: drain pending first, then halve the critical
                        # eviction->L4 chain by splitting across S and V
                        if b == 0:
                            flush_l4(upto=mi - 1)
                        at = apool.tile([128, BW], ST)
                        nc.scalar.activation(
                            at[:, 0:HWB], ps3[:, 0:HWB], Relu,
                            bias=b3t[:, mi : mi + 1],
                        )
                        nc.vector.tensor_scalar(
                            at[:, HWB:BW], ps3[:, HWB:BW],
                            b3t[:, mi : mi + 1], 0.0,
                            op0=mybir.AluOpType.add, op1=mybir.AluOpType.max,
                        )
                        w4s = w4a[:, mi * 128 : (mi + 1) * 128]
                        nc.tensor.matmul(
                            po[b][:, 0:HWB], w4s, at[:, 0:HWB],
                            start=False, stop=True,
                        )
                        nc.tensor.matmul(
                            po[b][:, HWB:BW], w4s, at[:, HWB:BW],
                            start=False, stop=True,
                        )
                        continue
                    if b == NBC - 1:
                        flush_l4(upto=mi - 1)
                    at = apool.tile([128, BW], ST)
                    if i3 % 5 < 3:
                        nc.scalar.activation(
                            at[:, :], ps3[:, :], Relu, bias=b3t[:, mi : mi + 1]
                        )
                    else:
                        nc.vector.tensor_scalar(
                            at[:, :], ps3[:, :], b3t[:, mi : mi + 1], 0.0,
                            op0=mybir.AluOpType.add, op1=mybir.AluOpType.max,
                        )
                    pend.append(
                        (mi, b, w4a[:, mi * 128 : (mi + 1) * 128], at)
                    )
                    if mi == NM3 - 1:
                        # drain b's L4 immediately: po[0] stops (and its
                        # eviction+DMA start) while b=1 is still in flight
                        flush_l4()
            # split final eviction + DMA per b-chunk, on separate engines
            nc.scalar.activation(
                outsb[:, 0:BW], po[0][:, :], Ident, bias=b4t[:, 0:1]
            )
            nc.sync.dma_start(out_d[:, 0:BW], outsb[:, 0:BW])
            nc.vector.tensor_scalar_add(
                outsb[:, BW:BC], po[1][:, :], b4t[:, 0:1]
            )
            nc.sync.dma_start(out_d[:, BW:BC], outsb[:, BW:BC])

    nc.compile()
    return nc


def _build_v9(mode: str):
    """v3 + fp8(e4m3) DoubleRow for the first NP8 k-tile PAIRS of L1.

    DoubleRow contracts two k-tiles per pass (2x PE rate), so k-tiles
    0..2*NP8-1 of the G contraction run at half cycles.  Error scales as
    ~sqrt(alpha): NP8=18 (alpha=0.23) measures ~1.7e-2 on CPU sim vs the
    2e-2 gate.  Scales: x*2^-1 and W1*2^6 (exact pow2, applied to BOTH
    the fp8 and bf16 slices) put both operands mid-range in e4m3; the
    2^5 product factor costs zero instructions: host pre-scales b1*2^5
    (h1a holds 2^5*h1) and W2*2^-5 (z2 exact, downstream untouched).
    """
    import concourse.bacc as bacc
    import concourse.tile as tile
    import concourse.mybir as mybir

    FP = mybir.dt.float32
    ST = mybir.dt.bfloat16
    F8 = mybir.dt.float8e4
    DR = mybir.MatmulPerfMode.DoubleRow

    # v10: L4 off the PE.  W3 host-permuted to [H2, A, P] so each L3
    # chunk-matmul yields psum[p(rotein), b] for ONE a; the W4 dot over a
    # becomes a per-a fused multiply-accumulate on Vector/GpSimd
    # (acc = t_a*w4[:,a] + acc), killing all 128 L4 matmuls (65.5K PE
    # cycles = ~27us).  Eviction instructions are unchanged (b3[:,a] is
    # per-partition in this layout too).
    L4NEW = mode.startswith("v10")
    _parts = mode.split("_")
    NP8 = int(_parts[1]) if len(_parts) > 1 else 20
    NPE = int(_parts[2]) if len(_parts) > 2 else 30  # a's reduced on PE (diag)
    NKF8 = 2 * NP8              # fp8 k-tiles
    NKB = NKP - NKF8            # bf16 k-tiles

    # fp8 phase: groups in PAIR units (each pair = DMA bytes of one bf16
    # tile, same compute); then bf16 phase continues the v3 ramp.
    sizes8 = []
    _ramp = [1, 1, 2, 3, 4, 4]
    _r = NP8
    for s in _ramp:
        s = min(s, _r)
        if s == 0:
            break
        sizes8.append(s)
        _r -= s
    while _r:
        s = min(4, _r)
        sizes8.append(s)
        _r -= s
    kgroups8 = []
    _k0 = 0
    for s in sizes8:
        kgroups8.append((_k0, s))
        _k0 += s
    assert _k0 == NP8
    KG8 = max(sizes8)

    KGv = 8
    sizesb = [5, 6] + [8] * ((NKB - 11) // 8)
    _rem = NKB - sum(sizesb)
    if _rem:
        sizesb.append(_rem)
    kgroupsb = []
    _k0 = 0
    for s in sizesb:
        kgroupsb.append((_k0, s))
        _k0 += s
    assert _k0 == NKB
    NGRP = len(kgroups8) + len(kgroupsb)
    PREFETCH_AT = NGRP - 5      # W3/W4 prefetch DMA issue point (group idx)
    xbufs = 4

    Relu = mybir.ActivationFunctionType.Relu
    Ident = mybir.ActivationFunctionType.Identity

    nc = bacc.Bacc(
        "TRN2", target_bir_lowering=False, debug=False, num_devices=NCORES
    )

    x8_d = nc.dram_tensor("x8i", [128, NKF8 * BC], F8, kind="ExternalInput")
    w18_d = nc.dram_tensor("w18i", [128, NKF8 * H1], F8, kind="ExternalInput")
    xT_d = nc.dram_tensor("xTi", [128, NKB * BC], ST, kind="ExternalInput")
    w1t_d = nc.dram_tensor("w1ti", [128, NKB * H1], ST, kind="ExternalInput")
    b1r_d = nc.dram_tensor("b1r", [128, NM1], FP, kind="ExternalInput")
    w2i_d = nc.dram_tensor("w2i", [128, NK2 * H2], ST, kind="ExternalInput")
    b2r_d = nc.dram_tensor("b2r", [128, NM2], FP, kind="ExternalInput")
    w3f_d = nc.dram_tensor("w3f", [H2, PA], ST, kind="ExternalInput")
    if L4NEW:
        b3r_d = nc.dram_tensor("b3r", [128, A], FP, kind="ExternalInput")
        w4r_d = nc.dram_tensor("w4r", [128, A], FP, kind="ExternalInput")
        if NPE:
            w4d_d = nc.dram_tensor(
                "w4d", [128, NPE * 128], ST, kind="ExternalInput"
            )
    else:
        b3r_d = nc.dram_tensor("b3r", [128, NM3], FP, kind="ExternalInput")
        w4i_d = nc.dram_tensor("w4i", [128, NM3 * 128], ST, kind="ExternalInput")
    b4r_d = nc.dram_tensor("b4r", [128, 1], FP, kind="ExternalInput")
    out_d = nc.dram_tensor("out", [P, BC], FP, kind="ExternalOutput")

    with tile.TileContext(nc) as tc:
        with (
            tc.tile_pool(name="const", bufs=1) as const,
            tc.tile_pool(name="wpre", bufs=1) as wpre,
            tc.tile_pool(name="h1", bufs=1) as h1pool,
            tc.tile_pool(name="h2", bufs=1) as h2pool,
            tc.tile_pool(name="osb", bufs=1) as opool,
            tc.tile_pool(name="x8p", bufs=3) as x8pool,
            tc.tile_pool(name="w8p", bufs=3) as w8pool,
            tc.tile_pool(name="xp", bufs=xbufs) as xpool,
            tc.tile_pool(name="w1p", bufs=xbufs) as w1pool,
            tc.tile_pool(name="ap", bufs=12) as apool,
            tc.tile_pool(name="ps", bufs=1, space="PSUM") as ps,
        ):
            b1t = const.tile([128, NM1], FP)
            nc.scalar.dma_start(b1t[:], b1r_d[:])
            b2t = const.tile([128, NM2], FP)
            nc.scalar.dma_start(b2t[:], b2r_d[:])
            if L4NEW:
                b3t = const.tile([128, A], FP)
                nc.scalar.dma_start(b3t[:], b3r_d[:])
                w4t = const.tile([128, A], FP)
                nc.scalar.dma_start(w4t[:], w4r_d[:])
                if NPE:
                    # DMA issued at PREFETCH_AT (startup HBM is ramp-critical)
                    w4dt = const.tile([128, NPE * 128], ST)
            else:
                b3t = const.tile([128, NM3], FP)
                nc.scalar.dma_start(b3t[:], b3r_d[:])
            b4t = const.tile([128, 1], FP)
            nc.scalar.dma_start(b4t[:], b4r_d[:])
            w2a = wpre.tile([128, NK2 * H2], ST)
            nc.gpsimd.dma_start(w2a[:], w2i_d[:])
            w3a = wpre.tile([128, PA], ST)
            w3b = wpre.tile([128, PA], ST)
            if not L4NEW:
                w4a = wpre.tile([128, NM3 * 128], ST)

            h1a = h1pool.tile([128, NM1 * BC], ST)
            h2a = h2pool.tile([128, NM2 * BC], ST)
            outsb = opool.tile([128, BC], FP)
            if L4NEW:
                F16 = mybir.dt.float16
                accv = [
                    opool.tile([128, BW], F16, name=f"accv{g}")
                    for g in range(NBC)
                ]
                accg = [
                    opool.tile([128, BW], F16, name=f"accg{g}")
                    for g in range(NBC)
                ]

            pst = [
                ps.tile([128, BW], FP, name=f"pst{i}", tag=f"pst{i}")
                for i in range(NM1 * NBC)
            ]

            # ---- L1 phase A: fp8 DoubleRow over k-tiles 0..NKF8-1 ----
            gi = 0
            for (p0, psz) in kgroups8:
                x8t = x8pool.tile([128, KG8 * 2 * BC], F8)
                w8t = w8pool.tile([128, KG8 * 2 * H1], F8)
                nc.sync.dma_start(
                    w8t[:, : psz * 2 * H1],
                    w18_d[:, p0 * 2 * H1 : (p0 + psz) * 2 * H1],
                )
                nc.sync.dma_start(
                    x8t[:, : psz * 2 * BC],
                    x8_d[:, p0 * 2 * BC : (p0 + psz) * 2 * BC],
                )
                for m in range(NM1):
                    for b in range(NBC):
                        for pp in range(psz):
                            pr = p0 + pp
                            w3d = w8t[
                                :, 2 * pp * H1 : (2 * pp + 2) * H1
                            ].rearrange("p (two h) -> p two h", two=2)
                            x3d = x8t[
                                :, 2 * pp * BC : (2 * pp + 2) * BC
                            ].rearrange("p (two j) -> p two j", two=2)
                            nc.tensor.matmul(
                                pst[m * NBC + b][:, :],
                                w3d[:, :, m * 128 : (m + 1) * 128],
                                x3d[:, :, b * BW : (b + 1) * BW],
                                start=(pr == 0),
                                stop=False,
                                perf_mode=DR,
                            )
                gi += 1

            # ---- L1 phase B: bf16 over k-tiles NKF8..NKP-1 ----
            for (k0, gsz) in kgroupsb:
                if gi == PREFETCH_AT:
                    nc.sync.dma_start(w3a[:], w3f_d[0:128, :])
                    nc.sync.dma_start(w3b[:], w3f_d[128:256, :])
                    if L4NEW:
                        if NPE:
                            nc.scalar.dma_start(w4dt[:], w4d_d[:])
                    else:
                        nc.sync.dma_start(w4a[:], w4i_d[:])
                wt = w1pool.tile([128, KGv * H1], ST)
                xt = xpool.tile([128, KGv * BC], ST)
                nc.sync.dma_start(
                    wt[:, : gsz * H1], w1t_d[:, k0 * H1 : (k0 + gsz) * H1]
                )
                nc.sync.dma_start(
                    xt[:, : gsz * BC], xT_d[:, k0 * BC : (k0 + gsz) * BC]
                )
                for m in range(NM1):
                    for b in range(NBC):
                        for kk in range(gsz):
                            ki = NKF8 + k0 + kk
                            nc.tensor.matmul(
                                pst[m * NBC + b][:, :],
                                wt[:, kk * H1 + m * 128 : kk * H1 + (m + 1) * 128],
                                xt[:, kk * BC + b * BW : kk * BC + (b + 1) * BW],
                                start=False,
                                stop=(ki == NKP - 1),
                            )
                gi += 1
            # L1 evictions: m-major so bank m*2+b frees in the order L2
            # consumes h1a m-blocks (GpSimd cannot read PSUM, so S/V only).
            for m in range(NM1):
                for b in range(NBC):
                    i = m * NBC + b
                    c0 = m * BC + b * BW
                    if i % 2 == 0:
                        nc.scalar.activation(
                            h1a[:, c0 : c0 + BW], pst[i][:, :], Relu,
                            bias=b1t[:, m : m + 1],
                        )
                    else:
                        nc.vector.tensor_scalar(
                            h1a[:, c0 : c0 + BW], pst[i][:, :],
                            b1t[:, m : m + 1], 0.0,
                            op0=mybir.AluOpType.add, op1=mybir.AluOpType.max,
                        )

            # ---- L2: h2T = relu(W2T.T @ h1T + b2), K over H1 ----
            pst2 = [
                ps.tile([128, BW], FP, name=f"pst2_{i}", tag=f"pst{i}")
                for i in range(NM2 * NBC)
            ]
            for ki in range(NK2):
                for m in range(NM2):
                    for b in range(NBC):
                        nc.tensor.matmul(
                            pst2[m * NBC + b][:, :],
                            w2a[:, ki * H2 + m * 128 : ki * H2 + (m + 1) * 128],
                            h1a[:, ki * BC + b * BW : ki * BC + b * BW + BW],
                            start=(ki == 0),
                            stop=(ki == NK2 - 1),
                        )
            for b in range(NBC):
                for m in range(NM2):
                    c0 = m * BC + b * BW
                    if m % 2 == 0:
                        nc.scalar.activation(
                            h2a[:, c0 : c0 + BW], pst2[m * NBC + b][:, :], Relu,
                            bias=b2t[:, m : m + 1],
                        )
                    else:
                        nc.vector.tensor_scalar(
                            h2a[:, c0 : c0 + BW], pst2[m * NBC + b][:, :],
                            b2t[:, m : m + 1], 0.0,
                            op0=mybir.AluOpType.add, op1=mybir.AluOpType.max,
                        )

            if L4NEW:
                # ---- L3: per-a psum[p, b].  W4-reduction hybrid: most a's
                # via fp16 stt chains on Vector, every (A//NPE)-th a via a
                # diagonal-W4 matmul accumulating in PSUM (PE has slack,
                # Vector does not). Evictions 3:1 Scalar:Vector. ----
                rot6 = ["pst4", "pst6", "pst5", "pst7", "pst0", "pst2"]
                w3ab = [w3a, w3b]
                add, mult, mx = (
                    mybir.AluOpType.add, mybir.AluOpType.mult,
                    mybir.AluOpType.max,
                )
                pe_set = set(
                    a for a in range(A)
                    if (a + 1) * NPE // A > a * NPE // A
                ) if NPE else set()
                pe_idx = {a: j for j, a in enumerate(sorted(pe_set))}
                pe_first, pe_last = (min(pe_set), max(pe_set)) if NPE else (0, 0)
                po = [
                    ps.tile([128, BW], FP, name=f"po{i}", tag=f"pst{1 + 2 * i}")
                    for i in range(NBC)
                ] if NPE else None
                seedv = [True] * NBC
                seedg = [True] * NBC
                stt_cnt = [0] * NBC
                last_stt = max(a for a in range(A) if a not in pe_set)
                pend = []

                def flush_pe(upto=None):
                    keep = []
                    for (pa, pg, pat) in pend:
                        if upto is not None and pa > upto:
                            keep.append((pa, pg, pat))
                            continue
                        j = pe_idx[pa]
                        nc.tensor.matmul(
                            po[pg][:, :],
                            w4dt[:, j * 128 : (j + 1) * 128],
                            pat[:, :],
                            start=(pa == pe_first),
                            stop=(pa == pe_last),
                        )
                    pend[:] = keep

                ri = 0
                for a in range(A):
                    for g in range(NBC):
                        i3 = a * NBC + g
                        ps3 = ps.tile(
                            [128, BW], FP, name=f"ps3_{i3}", tag=rot6[ri % 6]
                        )
                        ri += 1
                        for k in range(NK3):
                            nc.tensor.matmul(
                                ps3[:, :],
                                w3ab[k][:, a * 128 : (a + 1) * 128],
                                h2a[:, k * BC + g * BW : k * BC + g * BW + BW],
                                start=(k == 0),
                                stop=(k == NK3 - 1),
                            )
                        if NPE and g == NBC - 1:
                            flush_pe(upto=a - 1)
                        at = apool.tile([128, BW], ST)
                        if i3 % 4 != 3:
                            nc.scalar.activation(
                                at[:, :], ps3[:, :], Relu,
                                bias=b3t[:, a : a + 1],
                            )
                        else:
                            nc.vector.tensor_scalar(
                                at[:, :], ps3[:, :], b3t[:, a : a + 1], 0.0,
                                op0=add, op1=mx,
                            )
                        if a in pe_set:
                            pend.append((a, g, at))
                            continue
                        w4s = w4t[:, a : a + 1]
                        stt_cnt[g] += 1
                        if stt_cnt[g] % 2 == 1:
                            if seedv[g]:
                                seedv[g] = False
                                nc.vector.tensor_scalar(
                                    accv[g][:, :], at[:, :], w4s, b4t[:, 0:1],
                                    op0=mult, op1=add,
                                )
                            else:
                                nc.vector.scalar_tensor_tensor(
                                    accv[g][:, :], at[:, :], w4s,
                                    accv[g][:, :], op0=mult, op1=add,
                                )
                        else:
                            if seedg[g]:
                                seedg[g] = False
                                nc.vector.tensor_scalar(
                                    accg[g][:, :], at[:, :], w4s, 0.0,
                                    op0=mult, op1=add,
                                )
                            else:
                                nc.vector.scalar_tensor_tensor(
                                    accg[g][:, :], at[:, :], w4s,
                                    accg[g][:, :], op0=mult, op1=add,
                                )
                        if a == last_stt:
                            # pre-merge the two stt chains while the PE is
                            # still streaming the remaining diag matmuls
                            nc.vector.tensor_add(
                                outsb[:, g * BW : (g + 1) * BW],
                                accv[g][:, :], accg[g][:, :],
                            )
                if NPE:
                    flush_pe()
                for g in range(NBC):
                    if NPE:
                        nc.vector.tensor_add(
                            outsb[:, g * BW : (g + 1) * BW],
                            outsb[:, g * BW : (g + 1) * BW],
                            po[g][:, :],
                        )
                    nc.sync.dma_start(
                        out_d[:, g * BW : (g + 1) * BW],
                        outsb[:, g * BW : (g + 1) * BW],
                    )
            else:
                # ---- L3+L4: aT chunks then block-diag W4 reduction ----
                po = [
                    ps.tile([128, BW], FP, name=f"po{i}", tag=f"pst{4 + i}")
                    for i in range(NBC)
                ]
                rot = ["pst6", "pst7", "pst2", "pst0"]
                w3ab = [w3a, w3b]
                pend = []

                def flush_l4(upto=None):
                    keep = []
                    for (pmi, pb, pw4, pat) in pend:
                        if upto is not None and pmi > upto:
                            keep.append((pmi, pb, pw4, pat))
                            continue
                        nc.tensor.matmul(
                            po[pb][:, :], pw4, pat[:, :],
                            start=(pmi == 0), stop=(pmi == NM3 - 1),
                        )
                    pend[:] = keep

                for mi in range(NM3):
                    for b in range(NBC):
                        i3 = mi * NBC + b
                        ps3 = ps.tile(
                            [128, BW], FP, name=f"ps3_{i3}", tag=rot[i3 % 4]
                        )
                        for k in range(NK3):
                            nc.tensor.matmul(
                                ps3[:, :],
                                w3ab[k][:, mi * 128 : (mi + 1) * 128],
                                h2a[:, k * BC + b * BW : k * BC + b * BW + BW],
                                start=(k == 0),
                                stop=(k == NK3 - 1),
                            )
                        if b == NBC - 1:
                            flush_l4(upto=mi - 1)
                        at = apool.tile([128, BW], ST)
                        if i3 % 5 < 3:
                            nc.scalar.activation(
                                at[:, :], ps3[:, :], Relu, bias=b3t[:, mi : mi + 1]
                            )
                        else:
                            nc.vector.tensor_scalar(
                                at[:, :], ps3[:, :], b3t[:, mi : mi + 1], 0.0,
                                op0=mybir.AluOpType.add, op1=mybir.AluOpType.max,
                            )
                        pend.append(
                            (mi, b, w4a[:, mi * 128 : (mi + 1) * 128], at)
                        )
                        if mi == NM3 - 1:
                            flush_l4()
                nc.scalar.activation(
                    outsb[:, 0:BW], po[0][:, :], Ident, bias=b4t[:, 0:1]
                )
                nc.sync.dma_start(out_d[:, 0:BW], outsb[:, 0:BW])
                nc.vector.tensor_scalar_add(
                    outsb[:, BW:BC], po[1][:, :], b4t[:, 0:1]
                )
                nc.sync.dma_start(out_d[:, BW:BC], outsb[:, BW:BC])

    nc.compile()
    return nc


def _get_nc(mode: str):
    if mode not in _CACHE:
        if mode.startswith("v9") or mode.startswith("v10"):
            _CACHE[mode] = _build_v9(mode)
        elif mode.startswith("v"):
            _CACHE[mode] = _build_v3(mode)
        else:
            _CACHE[mode] = _build(mode)
    return _CACHE[mode]


def _interleave_k(mat_gp: np.ndarray) -> np.ndarray:
    """[GP, F] -> [128, NKP*F] with col block ki = k-tile ki."""
    f = mat_gp.shape[1]
    return np.ascontiguousarray(
        mat_gp.reshape(NKP, 128, f).transpose(1, 0, 2).reshape(128, NKP * f)
    )


def _prep_inputs(x, W1, b1, W2, b2, W3, b3, W4, b4, mode="f32r"):
    f = np.float32
    if mode.startswith("bf16") or mode.startswith("v"):
        import ml_dtypes

        st = np.dtype(ml_dtypes.bfloat16)
    else:
        st = np.dtype(np.float32)
    ac = np.ascontiguousarray

    def cst(a):
        return a if a.dtype == st else a.astype(st)

    V9 = mode.startswith("v9") or mode.startswith("v10")
    V10 = mode.startswith("v10")
    if V9:
        import ml_dtypes

        f8 = np.dtype(ml_dtypes.float8_e4m3)
        NP8 = int(mode.split("_")[1]) if "_" in mode else 20
        NKF8 = 2 * NP8
        K8 = NKF8 * 128
        SX, SW, SB = 2.0 ** -1, 2.0 ** 6, 2.0 ** 5
    else:
        SX = SW = SB = 1.0
        K8 = NKF8 = 0

    x = np.asarray(x, f)
    xTp = np.zeros((GP, B), st)
    np.copyto(xTp[:G], cst(x.T * SX))                          # [GP, B]
    w1tp = np.zeros((GP, H1), st)
    np.copyto(w1tp[:G], cst(np.asarray(W1, f).T * SW))
    if V9:
        # fp8 region: k-tiles 0..NKF8-1 quantized e4m3 from the scaled f32
        xT8 = ac((x.T[:K8] * SX).astype(f8))                   # [K8, B]
        w18p = ac((np.asarray(W1, f).T[:K8] * SW).astype(f8))  # [K8, H1]
        w18i = ac(
            w18p.reshape(NKF8, 128, H1).transpose(1, 0, 2).reshape(128, NKF8 * H1)
        )
        xTp = xTp[K8:]                                         # bf16 region
        w1tp = w1tp[K8:]

        def _ik(mat):  # [nk*128, F] -> [128, nk*F]
            nk = mat.shape[0] // 128
            fdim = mat.shape[1]
            return np.ascontiguousarray(
                mat.reshape(nk, 128, fdim).transpose(1, 0, 2).reshape(128, nk * fdim)
            )

        w1ti = _ik(w1tp)
    else:
        w1ti = _interleave_k(w1tp)                             # [128, NKP*H1]
    b1r = ac(np.asarray(b1, f).reshape(NM1, 128).T * SB)       # [128, 4]
    w2T = cst(np.asarray(W2, f).T / SB)                        # [H1, H2]
    if mode.startswith("v"):
        # k-interleaved single-DMA layout: w2i[p, ki*H2+c] = W2T[ki*128+p, c]
        w2t = ac(w2T.reshape(NK2, 128, H2).transpose(1, 0, 2).reshape(128, NK2 * H2))
        w2key = "w2i"
    else:
        w2t = ac(w2T)
        w2key = "w2t"
    b2r = ac(np.asarray(b2, f).reshape(NM2, 128).T)            # [128, 2]
    if V10:
        # w3f[h, a*P + p] = W3[p, h, a]; b3/w4 land per-partition over p
        w3f = ac(cst(np.asarray(W3, f).transpose(1, 2, 0).reshape(H2, A * P)))
        b3r = ac(np.asarray(b3, f))                                # [128, 64]
        w4r = ac(np.asarray(W4, f))                                # [128, 64]
        b4r = ac(np.asarray(b4, f).reshape(128, 1))
        shared = {
            "w1ti": w1ti, "b1r": b1r, w2key: w2t, "b2r": b2r,
            "w3f": w3f, "b3r": b3r, "w4r": w4r, "b4r": b4r,
        }
        NPE = int(mode.split("_")[2]) if mode.count("_") > 1 else 30
        if NPE:
            # diag(W4[:, a_j]) for the PE-reduced a's (even spread over A)
            pe_as = [
                a for a in range(A) if (a + 1) * NPE // A > a * NPE // A
            ]
            w4d = np.zeros((128, NPE * 128), np.float32)
            for j, aj in enumerate(pe_as):
                w4d[np.arange(128), j * 128 + np.arange(128)] = np.asarray(
                    W4, f
                )[:, aj]
            shared["w4d"] = ac(cst(w4d))
    else:
        w3f = ac(cst(np.asarray(W3, f).transpose(1, 0, 2).reshape(H2, PA)))
        b3r = ac(np.asarray(b3, f).reshape(PA).reshape(NM3, 128).T)  # [128, 64]
        w4bd = np.zeros((PA, P), st)
        w4bd[np.arange(PA), np.arange(PA) // A] = cst(np.asarray(W4, f).reshape(PA))
        # k-tile-interleaved block-diag W4: w4i[p, mi*128+c] = w4bd[mi*128+p, c]
        w4i = ac(w4bd.reshape(NM3, 128, P).transpose(1, 0, 2).reshape(128, NM3 * P))
        b4r = ac(np.asarray(b4, f).reshape(128, 1))
        shared = {
            "w1ti": w1ti, "b1r": b1r, w2key: w2t, "b2r": b2r,
            "w3f": w3f, "b3r": b3r, "w4i": w4i, "b4r": b4r,
        }
    if V9:
        shared["w18i"] = w18i
    in_maps = []
    for c in range(NCORES):
        if V9:
            m = {
                "xTi": _ik(xTp[:, c * BC : (c + 1) * BC]),
                "x8i": _ik(xT8[:, c * BC : (c + 1) * BC]),
            }
        else:
            m = {"xTi": _interleave_k(xTp[:, c * BC : (c + 1) * BC])}
        m.update(shared)
        in_maps.append(m)
    return in_maps


def run_with_results(inputs: dict, trace: bool = False, mode: str | None = None):
    """Returns (full_output [B, P] float32, BassKernelResults)."""
    from concourse.bass_utils import run_bass_kernel_spmd

    if mode is None:
        mode = os.environ.get("CTP_MODE", "v10")
    nc = _get_nc(mode)
    in_maps = _prep_inputs(**inputs, mode=mode)
    res = run_bass_kernel_spmd(
        nc, in_maps, core_ids=list(range(NCORES)), trace=trace
    )
    out = np.empty((B, P), np.float32)
    for c in range(NCORES):
        out[c * BC : (c + 1) * BC, :] = res.results[c]["out"].T
    return out, res


def kernel(**inputs) -> np.ndarray:
    out, _ = run_with_results(inputs, trace=False)
    return out



# revision 47
# speedup vs baseline: 1.0040x; 1.0040x over previous
"""CTPNet Trainium2 kernel: 8-way data-parallel over the batch dim.

Net (per reference):
    h1 = relu(x @ W1.T + b1)            x:[B,G]  W1:[H1,G]   -> [B,512]
    h2 = relu(h1 @ W2.T + b2)           W2:[H2,H1]           -> [B,256]
    a  = relu(einsum(bh,pha->bpa) + b3) W3:[P,H2,A]          -> [B,P,A]
    out= einsum(bpa,pa->bp) + b4        W4:[P,A]             -> [B,P]

B,G,H1,H2,P,A = 8192, 20000, 512, 256, 128, 64

Strategy: shard x rows 8 ways (1024 rows/core), replicate weights.
All on-chip tensors live in the "transposed" layout (feature dim on
partitions, batch on the free axis) so every layer is a plain chain of
TensorE matmuls with K (contraction) on the partition dim:

    h1T[512,1024]  = W1T.T-tiles @ xT-tiles          (157 K-tiles over G)
    h2T[256,1024]  = W2T-tiles @ h1T
    aT  [8192,1024] = W3f-tiles @ h2T  (heads flattened to [H2, P*A])
    outT[128,1024] = W4bd-tiles @ aT   (block-diag W4 does the A-reduction)

Host pre-transposes weights/x so every DMA is contiguous; host
re-assembles out = concat(outT_c.T).  Biases are folded into the
PSUM->SBUF eviction (Relu with per-partition bias, split across
ScalarE/VectorE).

Default mode v10 (~336 us/core, fro rel err 1.79e-2 vs the f32
reference; v3 history at the bottom) = v3 + two structural changes:

(1) fp8 split-K L1 (_build_v9, NP8=20 DoubleRow pairs): k-tiles
    0..39 of the G contraction run as e4m3 DoubleRow matmuls (lhsT
    [128,2,128] / rhs [128,2,BW] via AP rearrange, 2 k-tiles per
    512-cycle pass = 2x rate), the remaining 117 tiles stay bf16 and
    accumulate into the same PSUM banks.  Scales x*2^-1, W1*2^6
    (exact pow2, both fp8 AND bf16 slices -> PSUM = 2^5*z1); the
    descale is free: host scales b1*2^5 (h1a = 2^5*h1) and W2*2^-5
    (z2 exact; downstream untouched).  Error scales ~sqrt(NP8):
    e4m3 RTN is ~4.5% rms/operand -> 3.44e-2 at full-K (why plain
    fp8 failed before); NP8=20 measures 1.79e-2 on HW (=CPU-sim
    prediction to 3 digits; PE upcasts e4m3->e6m3 exactly).  NP8=22
    -> 1.875e-2 also passes but leaves only 6% gate margin.
(2) L4 hybrid off the PE (mode v10, NPE=30): W3 host-permuted to
    [H2, A, P] so each L3 chunk-matmul emits psum[protein, b] for
    ONE a (same 131K PE cycles, same moving operands, b3[:,a]/w4[:,a]
    per-partition).  The W4 dot over a then leaves the PE: 34 a's
    accumulate via vector.scalar_tensor_tensor fp16 chains
    (acc = at*w4[:,a] + acc, two independent chains/b-chunk, b4
    seeds chain 0, pre-merged before the PE drains), 30 a's stay on
    the PE as diag(W4[:,a]) matmuls into 2 PSUM banks (pst1/pst3),
    evictions 3:1 Scalar:Vector.  Old block-diag L4 cost 65.5K PE
    cycles for 0.13 GFLOP; the hybrid costs 30.7K PE cycles + ~50us
    of otherwise-idle V/S.  NPE sweep: 22->347us 28->341 30->336.7
    36->340 44->340 64(v9, all-PE)->348; all-DVE (NPE=0, f32 acc)
    ->397us: scalar_tensor_tensor is a 2-port DVE op, 741ns/512cols
    REGARDLESS of dtype (2X_1PORT does not apply), and Scalar
    activation evictions measure 687ns, so S/V saturate if the PE
    share drops below ~30.  TensorScalarPtr (tensor_scalar with AP
    scalar, and scalar_tensor_tensor) is INVALID on GpSimd: walrus
    "Instruction engine check failed (Pool)" -- GpSimd cannot help.
    bf16 acc chains add 7e-3 error (too much); fp16 adds <1e-3.

Measured budget at 336us: PE active 318us (745.6K cycles: L1 561K,
L2 8K, L3 131K, L4-diag 31K; gapless but for ~3.5us of ramp gaps),
~12.5us NEFF/DMA startup before the first matmul, ~6us tail
(po-merge + out DMA + drain), throttle ~9us from the cold start.
W4d diag + W3/W4 prefetch DMAs issue at PREFETCH_AT (mid-L1):
putting the 1MB w4d DMA at t=0 cost ~2us of ramp.  fp8 ramp
[1,1,2,3,4,4]+[4...] with 3 bufs of 4-pair groups; the finer ramps
[1,1,2,2,3,3...] and [1,1,1,2,3,4...] both STALLED the PE ~3-6us
around t=18-22us (and re-throttled the clock) -- do not re-tune
the ramp by +-1 groups, measured deltas are +-2.5us run noise.

v3 history (~379 us/core, rel err 4.4e-3): bf16 compute, 8-k-tile
DMA groups, single PSUM pool with per-bank tag chaining across
L1->L2->L3/L4, W2/W3/W4 fully prefetched into SBUF during L1,
consumer-ordered evictions, software-pipelined L4, split tail
eviction+DMA; PE gapless at 2.37 GHz for ~358 of the 379.  Dead
ends kept as modes: HAM warm-up dummies (v4), split first-tile DMA
(v5/v7 -- locks the clock at 1.98 GHz all run!), 16-tile groups
(v6), tail region-split (v8).  CTP_MODE=f32r: f32r end-to-end
(~494 us, DMA-bound).
"""

import os
import numpy as np

B, G, H1, H2, P, A = 8192, 20000, 512, 256, 128, 64
NCORES = 8
BC = B // NCORES            # 1024 batch rows per core
NBC = 2                     # b-chunks per core
BW = BC // NBC              # 512 (one PSUM bank / max fp32 moving free dim)
KT = 128
NKP = 157                   # K-tiles over G (156 full + one K=32 tail)
GP = NKP * KT               # 20096 (zero-padded from 20000)
KG = 4                      # max k-tiles per DMA group
# ramp-up schedule: small first chunks so the PE starts ~10us sooner
_sizes = [1, 1, 2] + [KG] * ((NKP - 4) // KG) + (
    [NKP - 4 - KG * ((NKP - 4) // KG)] if (NKP - 4) % KG else []
)
KGROUPS = []
_k = 0
for _s in _sizes:
    KGROUPS.append((_k, _s))
    _k += _s
assert _k == NKP
PA = P * A                  # 8192 flattened head outputs
NM3 = PA // 128             # 64 pa-chunks
NM1 = H1 // 128             # 4
NK2 = H1 // 128             # 4
NM2 = H2 // 128             # 2
NK3 = H2 // 128             # 2

_CACHE = {}


def _build(mode: str):
    """Build + compile the per-core Bass graph. mode: 'f32r' | 'f32' | 'bf16'."""
    import concourse.bacc as bacc
    import concourse.tile as tile
    import concourse.mybir as mybir
    from contextlib import ExitStack

    FP = mybir.dt.float32
    # ST: storage dtype of streamed/intermediate SBUF tiles (and big DRAM ins)
    ST = mybir.dt.bfloat16 if mode.startswith("bf16") else (
        mybir.dt.float32r if mode == "f32r" else mybir.dt.float32
    )
    DEEP = 6 if mode in ("bf16x", "bf16y", "bf16w") else 4
    PS3B = 5 if mode == "bf16y" else 4
    WARM = mode == "bf16w"  # HAM keep-warm dummy matmuls
    L4LAG = 2 if mode == "bf16w" else 1
    CHAIN = mode == "bf16t"  # single PSUM pool, per-bank tag chaining
    if mode in ("bf16v", "bf16u", "bf16t", "bf16s"):
        # fewer DMA groups: halves the per-group first-matmul sem-wait tax
        KGv = 16 if mode == "bf16s" else 8
        if mode == "bf16s":
            # halve the group count again: fewer first-matmul sem-wait taxes
            sizes_v = [1, 1, 2, 4, 8] + [16] * 8 + [13]
        elif mode == "bf16u":
            # gentler ramp + deeper buffers: kill the early-L1 DMA-pacing
            # stalls seen at t=17-31us in the bf16v trace
            sizes_v = [1, 1, 2, 4, 4] + [8] * 18 + [1]
        else:
            sizes_v = [1, 1, 2, 4] + [8] * 18 + [5]  # bf16v/bf16t
        kgroups = []
        kk0 = 0
        for sz in sizes_v:
            kgroups.append((kk0, sz))
            kk0 += sz
        assert kk0 == NKP
        xbufs = 4 if mode == "bf16u" else 3
        MIB = 4   # heads-chunks per W3/W4 load in the tail
    else:
        KGv = KG
        kgroups = KGROUPS
        xbufs = 6 if mode.startswith("bf16") else 4
        MIB = 1

    def mc(ap):
        return ap

    nc = bacc.Bacc(
        "TRN2", target_bir_lowering=False, debug=False, num_devices=NCORES
    )

    # k-tile-interleaved layouts: col block ki holds k-tile ki
    # xTi[p, ki*BC + j]  = x_core[j, ki*128 + p]   (zero-padded past G)
    # w1ti[p, ki*H1 + h] = W1[h, ki*128 + p]
    xT_d = nc.dram_tensor("xTi", [128, NKP * BC], ST, kind="ExternalInput")
    w1t_d = nc.dram_tensor("w1ti", [128, NKP * H1], ST, kind="ExternalInput")
    b1r_d = nc.dram_tensor("b1r", [128, NM1], FP, kind="ExternalInput")
    w2t_d = nc.dram_tensor("w2t", [H1, H2], ST, kind="ExternalInput")
    b2r_d = nc.dram_tensor("b2r", [128, NM2], FP, kind="ExternalInput")
    w3f_d = nc.dram_tensor("w3f", [H2, PA], ST, kind="ExternalInput")
    b3r_d = nc.dram_tensor("b3r", [128, NM3], FP, kind="ExternalInput")
    w4i_d = nc.dram_tensor("w4i", [128, NM3 * 128], ST, kind="ExternalInput")
    b4r_d = nc.dram_tensor("b4r", [128, 1], FP, kind="ExternalInput")
    out_d = nc.dram_tensor("out", [P, BC], FP, kind="ExternalOutput")

    Relu = mybir.ActivationFunctionType.Relu

    with tile.TileContext(nc) as tc:
        with (
            tc.tile_pool(name="const", bufs=1) as const,
            tc.tile_pool(name="h1", bufs=1) as h1pool,
            tc.tile_pool(name="h2", bufs=1) as h2pool,
            tc.tile_pool(name="osb", bufs=1) as opool,
            tc.tile_pool(name="xp", bufs=xbufs) as xpool,
            tc.tile_pool(name="w1p", bufs=xbufs) as w1pool,
        ):
            b1t = const.tile([128, NM1], FP)
            nc.scalar.dma_start(b1t[:], b1r_d[:])
            b2t = const.tile([128, NM2], FP)
            nc.scalar.dma_start(b2t[:], b2r_d[:])
            b3t = const.tile([128, NM3], FP)
            nc.scalar.dma_start(b3t[:], b3r_d[:])
            b4t = const.tile([128, 1], FP)
            nc.scalar.dma_start(b4t[:], b4r_d[:])

            # h1T as [128, m(4) x b(1024)]; col m*BC + j holds h1[m*128+p, j]
            h1a = h1pool.tile([128, NM1 * BC], ST)
            h2a = h2pool.tile([128, NM2 * BC], ST)
            outsb = opool.tile([128, BC], FP)
            warmt = None
            if WARM:
                warmt = const.tile([128, 64], ST)
                nc.vector.memset(warmt[:], 0.0)

            def warm_mms(ps_ap, n):
                # tiny matmuls on zeros: keep the PE-HAM activity window
                # busy through a stretch where the real stream would idle
                # (idle > ~3.4us rethrottles the PE clock to 1.2 GHz).
                # The consumer's start=True overwrites the garbage.
                for _ in range(n):
                    nc.tensor.matmul(
                        ps_ap[0:64, 0:64], warmt[:, 0:64], warmt[:, 0:64],
                        start=True, stop=True,
                    )

            # ---- L1: h1T = relu(W1T.T @ xT + b1), K over G ----
            pse = ExitStack()
            if True:
                ps1 = pse.enter_context(
                    tc.tile_pool(name="ps1", bufs=1, space="PSUM")
                )
                pst = [
                    ps1.tile([128, BW], FP, name=f"pst{i}", tag=f"pst{i}")
                    for i in range(NM1 * NBC)
                ]
                if WARM:
                    warm_mms(pst[0], 64)
                for (k0, gsz) in kgroups:
                    wt = w1pool.tile([128, KGv * H1], ST)
                    nc.sync.dma_start(
                        wt[:, : gsz * H1], w1t_d[:, k0 * H1 : (k0 + gsz) * H1]
                    )
                    xt = xpool.tile([128, KGv * BC], ST)
                    nc.sync.dma_start(
                        xt[:, : gsz * BC], xT_d[:, k0 * BC : (k0 + gsz) * BC]
                    )
                    # k-contiguous per PSUM bank: bank switches every gsz MMs
                    # (not every MM) to avoid psum-queue depth-cycling stalls
                    for m in range(NM1):
                        for b in range(NBC):
                            for kk in range(gsz):
                                ki = k0 + kk
                                nc.tensor.matmul(
                                    pst[m * NBC + b][:, :],
                                    mc(wt[:, kk * H1 + m * 128 : kk * H1 + (m + 1) * 128]),
                                    mc(xt[:, kk * BC + b * BW : kk * BC + (b + 1) * BW]),
                                    start=(ki == 0),
                                    stop=(ki == NKP - 1),
                                )
                for m in range(NM1):
                    for b in range(NBC):
                        c0 = m * BC + b * BW
                        if (m * NBC + b) % 2 == 0:
                            nc.scalar.activation(
                                h1a[:, c0 : c0 + BW],
                                pst[m * NBC + b][:, :],
                                Relu,
                                bias=b1t[:, m : m + 1],
                            )
                        else:
                            nc.vector.tensor_scalar(
                                h1a[:, c0 : c0 + BW],
                                pst[m * NBC + b][:, :],
                                b1t[:, m : m + 1],
                                0.0,
                                op0=mybir.AluOpType.add,
                                op1=mybir.AluOpType.max,
                            )

            # ---- L2: h2T = relu(W2T.T @ h1T + b2), K over H1 ----
            # CHAIN: L2 accumulators reuse L1's banks per-tag, so allocation
            # waits only for that bank's eviction -- not the whole pool close
            with tc.tile_pool(name="w2p", bufs=NK2) as w2pool:
                if CHAIN:
                    ps2 = ps1
                else:
                    pse.close()
                    pse = ExitStack()
                    ps2 = pse.enter_context(
                        tc.tile_pool(name="ps2", bufs=1, space="PSUM")
                    )
                pst2 = [
                    ps2.tile(
                        [128, BW], FP, name=f"pst2_{i}",
                        tag=(f"pst{i}" if CHAIN else f"pst2_{i}"),
                    )
                    for i in range(NM2 * NBC)
                ]
                if WARM:
                    warm_mms(pst2[0], 24)
                for ki in range(NK2):
                    w2t = w2pool.tile([128, H2], ST)
                    nc.scalar.dma_start(w2t[:], w2t_d[ki * 128 : (ki + 1) * 128, :])
                    for m in range(NM2):
                        for b in range(NBC):
                            nc.tensor.matmul(
                                pst2[m * NBC + b][:, :],
                                mc(w2t[:, m * 128 : (m + 1) * 128]),
                                mc(h1a[:, ki * BC + b * BW : ki * BC + b * BW + BW]),
                                start=(ki == 0),
                                stop=(ki == NK2 - 1),
                            )
                for m in range(NM2):
                    for b in range(NBC):
                        c0 = m * BC + b * BW
                        if (m * NBC + b) % 2 == 0:
                            nc.scalar.activation(
                                h2a[:, c0 : c0 + BW],
                                pst2[m * NBC + b][:, :],
                                Relu,
                                bias=b2t[:, m : m + 1],
                            )
                        else:
                            nc.vector.tensor_scalar(
                                h2a[:, c0 : c0 + BW],
                                pst2[m * NBC + b][:, :],
                                b2t[:, m : m + 1],
                                0.0,
                                op0=mybir.AluOpType.add,
                                op1=mybir.AluOpType.max,
                            )

            # ---- L3+L4: aT chunks then block-diag W4 reduction ----
            with (
                tc.tile_pool(name="w3p", bufs=DEEP) as w3pool,
                tc.tile_pool(name="w4p", bufs=DEEP) as w4pool,
                tc.tile_pool(name="ap", bufs=DEEP) as apool,
            ):
                if CHAIN:
                    ps4 = ps1
                    ps3pool = ps1
                else:
                    pse.close()
                    pse = ExitStack()
                    ps4 = pse.enter_context(
                        tc.tile_pool(name="ps4", bufs=1, space="PSUM")
                    )
                    ps3pool = pse.enter_context(
                        tc.tile_pool(name="ps3", bufs=PS3B, space="PSUM")
                    )
                po = [
                    ps4.tile(
                        [128, BW], FP, name=f"po{i}",
                        tag=(f"pst{4 + i}" if CHAIN else f"po{i}"),
                    )
                    for i in range(NBC)
                ]
                if WARM:
                    warm3 = ps3pool.tile([128, BW], FP, name="warm3", tag="ps3")
                    warm_mms(warm3, 16)
                # software-pipelined: L4 accumulation for step mi-1 is emitted
                # between step mi's L3 matmuls so the PSUM->SBUF eviction
                # latency never blocks the PE stream.
                pend = []  # (mi, b, w4t, at) awaiting their L4 matmul

                def flush_l4(upto=None):
                    keep = []
                    for (pmi, pb, pw4t, pat) in pend:
                        if upto is not None and pmi > upto:
                            keep.append((pmi, pb, pw4t, pat))
                            continue
                        nc.tensor.matmul(
                            po[pb][:, :],
                            mc(pw4t),
                            mc(pat[:, :]),
                            start=(pmi == 0),
                            stop=(pmi == NM3 - 1),
                        )
                    pend[:] = keep

                for mi in range(NM3):
                    ml = mi % MIB
                    if ml == 0:
                        w3t = w3pool.tile([128, MIB * H2], ST)
                        for k in range(NK3):
                            nc.sync.dma_start(
                                w3t[:, k * MIB * 128 : (k * MIB + MIB) * 128],
                                w3f_d[k * 128 : (k + 1) * 128,
                                      mi * 128 : (mi + MIB) * 128],
                            )
                        w4t = w4pool.tile([128, MIB * 128], ST)
                        nc.sync.dma_start(
                            w4t[:], w4i_d[:, mi * 128 : (mi + MIB) * 128]
                        )
                    mypend = []
                    for b in range(NBC):
                        _i3 = mi * NBC + b
                        ps3 = ps3pool.tile(
                            [128, BW], FP, name=f"ps3_{_i3}",
                            tag=(f"pst{_i3 % 4}" if CHAIN else "ps3"),
                        )
                        for k in range(NK3):
                            nc.tensor.matmul(
                                ps3[:, :],
                                mc(w3t[:, (k * MIB + ml) * 128 : (k * MIB + ml + 1) * 128]),
                                mc(h2a[:, k * BC + b * BW : k * BC + b * BW + BW]),
                                start=(k == 0),
                                stop=(k == NK3 - 1),
                            )
                        if b == NBC - 1:
                            # L4 for step mi-L4LAG: gives the eviction chain
                            # L4LAG steps of slack before the PE needs `at`
                            flush_l4(upto=mi - L4LAG)
                        at = apool.tile([128, BW], ST)
                        if (mi * NBC + b) % 5 < 3:
                            nc.scalar.activation(
                                at[:, :], ps3[:, :], Relu, bias=b3t[:, mi : mi + 1]
                            )
                        else:
                            # relu(x + b3) on VectorE: (x add b3) max 0
                            nc.vector.tensor_scalar(
                                at[:, :],
                                ps3[:, :],
                                b3t[:, mi : mi + 1],
                                0.0,
                                op0=mybir.AluOpType.add,
                                op1=mybir.AluOpType.max,
                            )
                        mypend.append((mi, b, w4t[:, ml * 128 : (ml + 1) * 128], at))
                    pend.extend(mypend)
                flush_l4()
                for b in range(NBC):
                    nc.vector.tensor_scalar_add(
                        outsb[:, b * BW : (b + 1) * BW], po[b][:, :], b4t[:, 0:1]
                    )
            pse.close()
            nc.sync.dma_start(out_d[:, :], outsb[:, :])

    nc.compile()
    return nc


def _build_v3(mode: str):
    """Scheduling-optimized bf16 build.

    vs bf16u: (1) single PSUM pool with per-bank tag chaining across
    L1->L2->L3/L4 so phase N+1's first matmul waits only on one bank's
    eviction, not a pool close; (2) W2/W3/W4 fully prefetched into SBUF
    during L1 (L3/L4 phase does zero DMA); (3) L1 evictions spread over
    Scalar/Vector/GpSimd, L2 evictions ordered b=0-first to unblock L3;
    (4) smoother DMA ramp; (5) split final eviction+DMA per b-chunk so
    the out DMA trigger latency overlaps the last evictions.
    """
    import concourse.bacc as bacc
    import concourse.tile as tile
    import concourse.mybir as mybir

    FP = mybir.dt.float32
    ST = mybir.dt.bfloat16
    if mode == "v6":
        KGv = 16
        sizes_v = [1, 1, 2, 3, 4, 5, 6] + [16] * 8 + [7]
        xbufs_n = 2
    else:
        KGv = 8
        sizes_v = [1, 1, 2, 3, 4, 5, 6] + [8] * 16 + [7]
        xbufs_n = 4
    assert sum(sizes_v) == NKP
    kgroups = []
    _k0 = 0
    for _s in sizes_v:
        kgroups.append((_k0, _s))
        _k0 += _s
    PREFETCH_AT = 18 if mode != "v6" else 11  # W3/W4 prefetch DMA issue point
    xbufs = xbufs_n
    V4 = mode in ("v4", "v5")
    G0S = mode in ("v4", "v5", "v7")
    TAILS = mode in ("v4", "v5", "v8")
    # v4's HAM warm-up experiment regressed: dummy matmuls are themselves
    # cold-clock-limited (53ns each) and delay the real stream, while the
    # cold-clock real start is well-matched to the slow early DMA ramp.
    NWARM = 130 if mode == "v4" else 0
    Relu = mybir.ActivationFunctionType.Relu
    Ident = mybir.ActivationFunctionType.Identity

    nc = bacc.Bacc(
        "TRN2", target_bir_lowering=False, debug=False, num_devices=NCORES
    )

    xT_d = nc.dram_tensor("xTi", [128, NKP * BC], ST, kind="ExternalInput")
    w1t_d = nc.dram_tensor("w1ti", [128, NKP * H1], ST, kind="ExternalInput")
    b1r_d = nc.dram_tensor("b1r", [128, NM1], FP, kind="ExternalInput")
    w2i_d = nc.dram_tensor("w2i", [128, NK2 * H2], ST, kind="ExternalInput")
    b2r_d = nc.dram_tensor("b2r", [128, NM2], FP, kind="ExternalInput")
    w3f_d = nc.dram_tensor("w3f", [H2, PA], ST, kind="ExternalInput")
    b3r_d = nc.dram_tensor("b3r", [128, NM3], FP, kind="ExternalInput")
    w4i_d = nc.dram_tensor("w4i", [128, NM3 * 128], ST, kind="ExternalInput")
    b4r_d = nc.dram_tensor("b4r", [128, 1], FP, kind="ExternalInput")
    out_d = nc.dram_tensor("out", [P, BC], FP, kind="ExternalOutput")

    with tile.TileContext(nc) as tc:
        with (
            tc.tile_pool(name="const", bufs=1) as const,
            tc.tile_pool(name="wpre", bufs=1) as wpre,
            tc.tile_pool(name="h1", bufs=1) as h1pool,
            tc.tile_pool(name="h2", bufs=1) as h2pool,
            tc.tile_pool(name="osb", bufs=1) as opool,
            tc.tile_pool(name="xp", bufs=xbufs) as xpool,
            tc.tile_pool(name="w1p", bufs=xbufs) as w1pool,
            tc.tile_pool(name="ap", bufs=8) as apool,
            tc.tile_pool(name="ps", bufs=1, space="PSUM") as ps,
        ):
            b1t = const.tile([128, NM1], FP)
            nc.scalar.dma_start(b1t[:], b1r_d[:])
            b2t = const.tile([128, NM2], FP)
            nc.scalar.dma_start(b2t[:], b2r_d[:])
            b3t = const.tile([128, NM3], FP)
            nc.scalar.dma_start(b3t[:], b3r_d[:])
            b4t = const.tile([128, 1], FP)
            nc.scalar.dma_start(b4t[:], b4r_d[:])
            # W2 is tiny and needed right after L1: load it up front on the
            # gpsimd queue (idle at start, doesn't contend with the x ramp).
            w2a = wpre.tile([128, NK2 * H2], ST)
            nc.gpsimd.dma_start(w2a[:], w2i_d[:])
            # W3/W4 prefetch buffers; DMAs issue mid-L1 (see loop below) so
            # the transfers slot into the x-stream's spare bandwidth.
            w3a = wpre.tile([128, PA], ST)
            w3b = wpre.tile([128, PA], ST)
            w4a = wpre.tile([128, NM3 * 128], ST)

            h1a = h1pool.tile([128, NM1 * BC], ST)
            h2a = h2pool.tile([128, NM2 * BC], ST)
            outsb = opool.tile([128, BC], FP)

            pst = [
                ps.tile([128, BW], FP, name=f"pst{i}", tag=f"pst{i}")
                for i in range(NM1 * NBC)
            ]

            if NWARM:
                # HAM warm-up: zero-dependency dummy matmuls run during the
                # startup DMA window so the PE clock is at full rate (and the
                # pipeline hot) when the first real k-tile lands.  Consumed
                # by nothing; pst[0]'s real k=0 matmul start=True overwrites.
                warmt = const.tile([128, 64], ST)
                nc.vector.memset(warmt[:], 0.0)
                for _ in range(NWARM):
                    nc.tensor.matmul(
                        pst[0][0:64, 0:64], warmt[:, 0:64], warmt[:, 0:64],
                        start=True, stop=True,
                    )

            # ---- L1: h1T = relu(W1T.T @ xT + b1), K over G ----
            for gi, (k0, gsz) in enumerate(kgroups):
                if gi == PREFETCH_AT:
                    nc.sync.dma_start(w3a[:], w3f_d[0:128, :])
                    nc.sync.dma_start(w3b[:], w3f_d[128:256, :])
                    nc.sync.dma_start(w4a[:], w4i_d[:])
                wt = w1pool.tile([128, KGv * H1], ST)
                xt = xpool.tile([128, KGv * BC], ST)
                if gi == 0 and G0S:
                    # split the first tile's transfers so matmul (m0,b0)
                    # waits on 160KB, not 384KB
                    for m in range(NM1):
                        nc.sync.dma_start(
                            wt[:, m * 128 : (m + 1) * 128],
                            w1t_d[:, m * 128 : (m + 1) * 128],
                        )
                    for b in range(NBC):
                        nc.sync.dma_start(
                            xt[:, b * BW : (b + 1) * BW],
                            xT_d[:, b * BW : (b + 1) * BW],
                        )
                else:
                    nc.sync.dma_start(
                        wt[:, : gsz * H1], w1t_d[:, k0 * H1 : (k0 + gsz) * H1]
                    )
                    nc.sync.dma_start(
                        xt[:, : gsz * BC], xT_d[:, k0 * BC : (k0 + gsz) * BC]
                    )
                for m in range(NM1):
                    for b in range(NBC):
                        for kk in range(gsz):
                            ki = k0 + kk
                            nc.tensor.matmul(
                                pst[m * NBC + b][:, :],
                                wt[:, kk * H1 + m * 128 : kk * H1 + (m + 1) * 128],
                                xt[:, kk * BC + b * BW : kk * BC + (b + 1) * BW],
                                start=(ki == 0),
                                stop=(ki == NKP - 1),
                            )
            # L1 evictions: m-major so bank m*2+b frees in the order L2
            # consumes h1a m-blocks (GpSimd cannot read PSUM, so S/V only).
            for m in range(NM1):
                for b in range(NBC):
                    i = m * NBC + b
                    c0 = m * BC + b * BW
                    if i % 2 == 0:
                        nc.scalar.activation(
                            h1a[:, c0 : c0 + BW], pst[i][:, :], Relu,
                            bias=b1t[:, m : m + 1],
                        )
                    else:
                        nc.vector.tensor_scalar(
                            h1a[:, c0 : c0 + BW], pst[i][:, :],
                            b1t[:, m : m + 1], 0.0,
                            op0=mybir.AluOpType.add, op1=mybir.AluOpType.max,
                        )

            # ---- L2: h2T = relu(W2T.T @ h1T + b2), K over H1 ----
            pst2 = [
                ps.tile([128, BW], FP, name=f"pst2_{i}", tag=f"pst{i}")
                for i in range(NM2 * NBC)
            ]
            for ki in range(NK2):
                for m in range(NM2):
                    for b in range(NBC):
                        nc.tensor.matmul(
                            pst2[m * NBC + b][:, :],
                            w2a[:, ki * H2 + m * 128 : ki * H2 + (m + 1) * 128],
                            h1a[:, ki * BC + b * BW : ki * BC + b * BW + BW],
                            start=(ki == 0),
                            stop=(ki == NK2 - 1),
                        )
            # L2 evictions b=0-first (L3's first k-pair reads both m-blocks
            # of b=0) and on separate engines so they land together.
            for b in range(NBC):
                for m in range(NM2):
                    c0 = m * BC + b * BW
                    if m % 2 == 0:
                        nc.scalar.activation(
                            h2a[:, c0 : c0 + BW], pst2[m * NBC + b][:, :], Relu,
                            bias=b2t[:, m : m + 1],
                        )
                    else:
                        nc.vector.tensor_scalar(
                            h2a[:, c0 : c0 + BW], pst2[m * NBC + b][:, :],
                            b2t[:, m : m + 1], 0.0,
                            op0=mybir.AluOpType.add, op1=mybir.AluOpType.max,
                        )

            # ---- L3+L4: aT chunks then block-diag W4 reduction ----
            po = [
                ps.tile([128, BW], FP, name=f"po{i}", tag=f"pst{4 + i}")
                for i in range(NBC)
            ]
            # ps3 rotation tags: banks that free earliest after L1/L2.
            rot = ["pst6", "pst7", "pst2", "pst0"]
            w3ab = [w3a, w3b]
            pend = []  # (mi, b, w4_ap, at) awaiting their L4 matmul

            def flush_l4(upto=None):
                keep = []
                for (pmi, pb, pw4, pat) in pend:
                    if upto is not None and pmi > upto:
                        keep.append((pmi, pb, pw4, pat))
                        continue
                    nc.tensor.matmul(
                        po[pb][:, :], pw4, pat[:, :],
                        start=(pmi == 0), stop=(pmi == NM3 - 1),
                    )
                pend[:] = keep

            HWB = BW // 2
            for mi in range(NM3):
                for b in range(NBC):
                    i3 = mi * NBC + b
                    ps3 = ps.tile(
                        [128, BW], FP, name=f"ps3_{i3}", tag=rot[i3 % 4]
                    )
                    for k in range(NK3):
                        nc.tensor.matmul(
                            ps3[:, :],
                            w3ab[k][:, mi * 128 : (mi + 1) * 128],
                            h2a[:, k * BC + b * BW : k * BC + b * BW + BW],
                            start=(k == 0),
                            stop=(k == NK3 - 1),
                        )
                    if mi == NM3 - 1 and TAILS:
                        # tail: drain pending first, then halve the critical
                        # eviction->L4 chain by splitting across S and V
                        if b == 0:
                            flush_l4(upto=mi - 1)
                        at = apool.tile([128, BW], ST)
                        nc.scalar.activation(
                            at[:, 0:HWB], ps3[:, 0:HWB], Relu,
                            bias=b3t[:, mi : mi + 1],
                        )
                        nc.vector.tensor_scalar(
                            at[:, HWB:BW], ps3[:, HWB:BW],
                            b3t[:, mi : mi + 1], 0.0,
                            op0=mybir.AluOpType.add, op1=mybir.AluOpType.max,
                        )
                        w4s = w4a[:, mi * 128 : (mi + 1) * 128]
                        nc.tensor.matmul(
                            po[b][:, 0:HWB], w4s, at[:, 0:HWB],
                            start=False, stop=True,
                        )
                        nc.tensor.matmul(
                            po[b][:, HWB:BW], w4s, at[:, HWB:BW],
                            start=False, stop=True,
                        )
                        continue
                    if b == NBC - 1:
                        flush_l4(upto=mi - 1)
                    at = apool.tile([128, BW], ST)
                    if i3 % 5 < 3:
                        nc.scalar.activation(
                            at[:, :], ps3[:, :], Relu, bias=b3t[:, mi : mi + 1]
                        )
                    else:
                        nc.vector.tensor_scalar(
                            at[:, :], ps3[:, :], b3t[:, mi : mi + 1], 0.0,
                            op0=mybir.AluOpType.add, op1=mybir.AluOpType.max,
                        )
                    pend.append(
                        (mi, b, w4a[:, mi * 128 : (mi + 1) * 128], at)
                    )
                    if mi == NM3 - 1:
                        # drain b's L4 immediately: po[0] stops (and its
                        # eviction+DMA start) while b=1 is still in flight
                        flush_l4()
            # split final eviction + DMA per b-chunk, on separate engines
            nc.scalar.activation(
                outsb[:, 0:BW], po[0][:, :], Ident, bias=b4t[:, 0:1]
            )
            nc.sync.dma_start(out_d[:, 0:BW], outsb[:, 0:BW])
            nc.vector.tensor_scalar_add(
                outsb[:, BW:BC], po[1][:, :], b4t[:, 0:1]
            )
            nc.sync.dma_start(out_d[:, BW:BC], outsb[:, BW:BC])

    nc.compile()
    return nc


def _build_v9(mode: str):
    """v3 + fp8(e4m3) DoubleRow for the first NP8 k-tile PAIRS of L1.

    DoubleRow contracts two k-tiles per pass (2x PE rate), so k-tiles
    0..2*NP8-1 of the G contraction run at half cycles.  Error scales as
    ~sqrt(alpha): NP8=18 (alpha=0.23) measures ~1.7e-2 on CPU sim vs the
    2e-2 gate.  Scales: x*2^-1 and W1*2^6 (exact pow2, applied to BOTH
    the fp8 and bf16 slices) put both operands mid-range in e4m3; the
    2^5 product factor costs zero instructions: host pre-scales b1*2^5
    (h1a holds 2^5*h1) and W2*2^-5 (z2 exact, downstream untouched).
    """
    import concourse.bacc as bacc
    import concourse.tile as tile
    import concourse.mybir as mybir

    FP = mybir.dt.float32
    ST = mybir.dt.bfloat16
    F8 = mybir.dt.float8e4
    DR = mybir.MatmulPerfMode.DoubleRow

    # v10: L4 off the PE.  W3 host-permuted to [H2, A, P] so each L3
    # chunk-matmul yields psum[p(rotein), b] for ONE a; the W4 dot over a
    # becomes a per-a fused multiply-accumulate on Vector/GpSimd
    # (acc = t_a*w4[:,a] + acc), killing all 128 L4 matmuls (65.5K PE
    # cycles = ~27us).  Eviction instructions are unchanged (b3[:,a] is
    # per-partition in this layout too).
    L4NEW = mode.startswith("v10")
    _parts = mode.split("_")
    NP8 = int(_parts[1]) if len(_parts) > 1 else 20
    NPE = int(_parts[2]) if len(_parts) > 2 else 30  # a's reduced on PE (diag)
    NKF8 = 2 * NP8              # fp8 k-tiles
    NKB = NKP - NKF8            # bf16 k-tiles

    # fp8 phase: groups in PAIR units (each pair = DMA bytes of one bf16
    # tile, same compute); then bf16 phase continues the v3 ramp.
    sizes8 = []
    _ramp = [1, 1, 2, 3, 4, 4]
    _r = NP8
    for s in _ramp:
        s = min(s, _r)
        if s == 0:
            break
        sizes8.append(s)
        _r -= s
    while _r:
        s = min(4, _r)
        sizes8.append(s)
        _r -= s
    kgroups8 = []
    _k0 = 0
    for s in sizes8:
        kgroups8.append((_k0, s))
        _k0 += s
    assert _k0 == NP8
    KG8 = max(sizes8)

    KGv = 8
    sizesb = [5, 6] + [8] * ((NKB - 11) // 8)
    _rem = NKB - sum(sizesb)
    if _rem:
        sizesb.append(_rem)
    kgroupsb = []
    _k0 = 0
    for s in sizesb:
        kgroupsb.append((_k0, s))
        _k0 += s
    assert _k0 == NKB
    NGRP = len(kgroups8) + len(kgroupsb)
    PREFETCH_AT = NGRP - 8      # W3/W4 prefetch DMA issue point (group idx)
    xbufs = 4

    Relu = mybir.ActivationFunctionType.Relu
    Ident = mybir.ActivationFunctionType.Identity

    nc = bacc.Bacc(
        "TRN2", target_bir_lowering=False, debug=False, num_devices=NCORES
    )

    x8_d = nc.dram_tensor("x8i", [128, NKF8 * BC], F8, kind="ExternalInput")
    w18_d = nc.dram_tensor("w18i", [128, NKF8 * H1], F8, kind="ExternalInput")
    xT_d = nc.dram_tensor("xTi", [128, NKB * BC], ST, kind="ExternalInput")
    w1t_d = nc.dram_tensor("w1ti", [128, NKB * H1], ST, kind="ExternalInput")
    b1r_d = nc.dram_tensor("b1r", [128, NM1], FP, kind="ExternalInput")
    w2i_d = nc.dram_tensor("w2i", [128, NK2 * H2], ST, kind="ExternalInput")
    b2r_d = nc.dram_tensor("b2r", [128, NM2], FP, kind="ExternalInput")
    w3f_d = nc.dram_tensor("w3f", [H2, PA], ST, kind="ExternalInput")
    if L4NEW:
        b3r_d = nc.dram_tensor("b3r", [128, A], FP, kind="ExternalInput")
        w4r_d = nc.dram_tensor("w4r", [128, A], FP, kind="ExternalInput")
        if NPE:
            w4d_d = nc.dram_tensor(
                "w4d", [128, NPE * 128], ST, kind="ExternalInput"
            )
    else:
        b3r_d = nc.dram_tensor("b3r", [128, NM3], FP, kind="ExternalInput")
        w4i_d = nc.dram_tensor("w4i", [128, NM3 * 128], ST, kind="ExternalInput")
    b4r_d = nc.dram_tensor("b4r", [128, 1], FP, kind="ExternalInput")
    out_d = nc.dram_tensor("out", [P, BC], FP, kind="ExternalOutput")

    with tile.TileContext(nc) as tc:
        with (
            tc.tile_pool(name="const", bufs=1) as const,
            tc.tile_pool(name="wpre", bufs=1) as wpre,
            tc.tile_pool(name="h1", bufs=1) as h1pool,
            tc.tile_pool(name="h2", bufs=1) as h2pool,
            tc.tile_pool(name="osb", bufs=1) as opool,
            tc.tile_pool(name="x8p", bufs=3) as x8pool,
            tc.tile_pool(name="w8p", bufs=3) as w8pool,
            tc.tile_pool(name="xp", bufs=xbufs) as xpool,
            tc.tile_pool(name="w1p", bufs=xbufs) as w1pool,
            tc.tile_pool(name="ap", bufs=8) as apool,
            tc.tile_pool(name="ps", bufs=1, space="PSUM") as ps,
        ):
            b1t = const.tile([128, NM1], FP)
            nc.scalar.dma_start(b1t[:], b1r_d[:])
            b2t = const.tile([128, NM2], FP)
            nc.scalar.dma_start(b2t[:], b2r_d[:])
            if L4NEW:
                b3t = const.tile([128, A], FP)
                nc.scalar.dma_start(b3t[:], b3r_d[:])
                w4t = const.tile([128, A], FP)
                nc.scalar.dma_start(w4t[:], w4r_d[:])
                if NPE:
                    # DMA issued at PREFETCH_AT (startup HBM is ramp-critical)
                    w4dt = const.tile([128, NPE * 128], ST)
            else:
                b3t = const.tile([128, NM3], FP)
                nc.scalar.dma_start(b3t[:], b3r_d[:])
            b4t = const.tile([128, 1], FP)
            nc.scalar.dma_start(b4t[:], b4r_d[:])
            w2a = wpre.tile([128, NK2 * H2], ST)
            nc.gpsimd.dma_start(w2a[:], w2i_d[:])
            w3a = wpre.tile([128, PA], ST)
            w3b = wpre.tile([128, PA], ST)
            if not L4NEW:
                w4a = wpre.tile([128, NM3 * 128], ST)

            h1a = h1pool.tile([128, NM1 * BC], ST)
            h2a = h2pool.tile([128, NM2 * BC], ST)
            outsb = opool.tile([128, BC], FP)
            if L4NEW:
                F16 = mybir.dt.float16
                accv = [
                    opool.tile([128, BW], F16, name=f"accv{g}")
                    for g in range(NBC)
                ]
                accg = [
                    opool.tile([128, BW], F16, name=f"accg{g}")
                    for g in range(NBC)
                ]

            pst = [
                ps.tile([128, BW], FP, name=f"pst{i}", tag=f"pst{i}")
                for i in range(NM1 * NBC)
            ]

            # ---- L1 phase A: fp8 DoubleRow over k-tiles 0..NKF8-1 ----
            gi = 0
            for (p0, psz) in kgroups8:
                x8t = x8pool.tile([128, KG8 * 2 * BC], F8)
                w8t = w8pool.tile([128, KG8 * 2 * H1], F8)
                nc.sync.dma_start(
                    w8t[:, : psz * 2 * H1],
                    w18_d[:, p0 * 2 * H1 : (p0 + psz) * 2 * H1],
                )
                nc.sync.dma_start(
                    x8t[:, : psz * 2 * BC],
                    x8_d[:, p0 * 2 * BC : (p0 + psz) * 2 * BC],
                )
                for m in range(NM1):
                    for b in range(NBC):
                        for pp in range(psz):
                            pr = p0 + pp
                            w3d = w8t[
                                :, 2 * pp * H1 : (2 * pp + 2) * H1
                            ].rearrange("p (two h) -> p two h", two=2)
                            x3d = x8t[
                                :, 2 * pp * BC : (2 * pp + 2) * BC
                            ].rearrange("p (two j) -> p two j", two=2)
                            nc.tensor.matmul(
                                pst[m * NBC + b][:, :],
                                w3d[:, :, m * 128 : (m + 1) * 128],
                                x3d[:, :, b * BW : (b + 1) * BW],
                                start=(pr == 0),
                                stop=False,
                                perf_mode=DR,
                            )
                gi += 1

            # ---- L1 phase B: bf16 over k-tiles NKF8..NKP-1 ----
            for (k0, gsz) in kgroupsb:
                if gi == PREFETCH_AT:
                    nc.sync.dma_start(w3a[:], w3f_d[0:128, :])
                    nc.sync.dma_start(w3b[:], w3f_d[128:256, :])
                    if L4NEW:
                        if NPE:
                            nc.scalar.dma_start(w4dt[:], w4d_d[:])
                    else:
                        nc.sync.dma_start(w4a[:], w4i_d[:])
                wt = w1pool.tile([128, KGv * H1], ST)
                xt = xpool.tile([128, KGv * BC], ST)
                nc.sync.dma_start(
                    wt[:, : gsz * H1], w1t_d[:, k0 * H1 : (k0 + gsz) * H1]
                )
                nc.sync.dma_start(
                    xt[:, : gsz * BC], xT_d[:, k0 * BC : (k0 + gsz) * BC]
                )
                for m in range(NM1):
                    for b in range(NBC):
                        for kk in range(gsz):
                            ki = NKF8 + k0 + kk
                            nc.tensor.matmul(
                                pst[m * NBC + b][:, :],
                                wt[:, kk * H1 + m * 128 : kk * H1 + (m + 1) * 128],
                                xt[:, kk * BC + b * BW : kk * BC + (b + 1) * BW],
                                start=False,
                                stop=(ki == NKP - 1),
                            )
                gi += 1
            # L1 evictions: m-major so bank m*2+b frees in the order L2
            # consumes h1a m-blocks (GpSimd cannot read PSUM, so S/V only).
            for m in range(NM1):
                for b in range(NBC):
                    i = m * NBC + b
                    c0 = m * BC + b * BW
                    if i % 2 == 0:
                        nc.scalar.activation(
                            h1a[:, c0 : c0 + BW], pst[i][:, :], Relu,
                            bias=b1t[:, m : m + 1],
                        )
                    else:
                        nc.vector.tensor_scalar(
                            h1a[:, c0 : c0 + BW], pst[i][:, :],
                            b1t[:, m : m + 1], 0.0,
                            op0=mybir.AluOpType.add, op1=mybir.AluOpType.max,
                        )

            # ---- L2: h2T = relu(W2T.T @ h1T + b2), K over H1 ----
            pst2 = [
                ps.tile([128, BW], FP, name=f"pst2_{i}", tag=f"pst{i}")
                for i in range(NM2 * NBC)
            ]
            for ki in range(NK2):
                for m in range(NM2):
                    for b in range(NBC):
                        nc.tensor.matmul(
                            pst2[m * NBC + b][:, :],
                            w2a[:, ki * H2 + m * 128 : ki * H2 + (m + 1) * 128],
                            h1a[:, ki * BC + b * BW : ki * BC + b * BW + BW],
                            start=(ki == 0),
                            stop=(ki == NK2 - 1),
                        )
            for b in range(NBC):
                for m in range(NM2):
                    c0 = m * BC + b * BW
                    if m % 2 == 0:
                        nc.scalar.activation(
                            h2a[:, c0 : c0 + BW], pst2[m * NBC + b][:, :], Relu,
                            bias=b2t[:, m : m + 1],
                        )
                    else:
                        nc.vector.tensor_scalar(
                            h2a[:, c0 : c0 + BW], pst2[m * NBC + b][:, :],
                            b2t[:, m : m + 1], 0.0,
                            op0=mybir.AluOpType.add, op1=mybir.AluOpType.max,
                        )

            if L4NEW:
                # ---- L3: per-a psum[p, b].  W4-reduction hybrid: most a's
                # via fp16 stt chains on Vector, every (A//NPE)-th a via a
                # diagonal-W4 matmul accumulating in PSUM (PE has slack,
                # Vector does not). Evictions 3:1 Scalar:Vector. ----
                rot6 = ["pst4", "pst6", "pst5", "pst7", "pst0", "pst2"]
                w3ab = [w3a, w3b]
                add, mult, mx = (
                    mybir.AluOpType.add, mybir.AluOpType.mult,
                    mybir.AluOpType.max,
                )
                pe_set = set(
                    a for a in range(A)
                    if (a + 1) * NPE // A > a * NPE // A
                ) if NPE else set()
                pe_idx = {a: j for j, a in enumerate(sorted(pe_set))}
                pe_first, pe_last = (min(pe_set), max(pe_set)) if NPE else (0, 0)
                po = [
                    ps.tile([128, BW], FP, name=f"po{i}", tag=f"pst{1 + 2 * i}")
                    for i in range(NBC)
                ] if NPE else None
                seedv = [True] * NBC
                seedg = [True] * NBC
                stt_cnt = [0] * NBC
                last_stt = max(a for a in range(A) if a not in pe_set)
                pend = []

                def flush_pe(upto=None):
                    keep = []
                    for (pa, pg, pat) in pend:
                        if upto is not None and pa > upto:
                            keep.append((pa, pg, pat))
                            continue
                        j = pe_idx[pa]
                        nc.tensor.matmul(
                            po[pg][:, :],
                            w4dt[:, j * 128 : (j + 1) * 128],
                            pat[:, :],
                            start=(pa == pe_first),
                            stop=(pa == pe_last),
                        )
                    pend[:] = keep

                ri = 0
                for a in range(A):
                    for g in range(NBC):
                        i3 = a * NBC + g
                        ps3 = ps.tile(
                            [128, BW], FP, name=f"ps3_{i3}", tag=rot6[ri % 6]
                        )
                        ri += 1
                        for k in range(NK3):
                            nc.tensor.matmul(
                                ps3[:, :],
                                w3ab[k][:, a * 128 : (a + 1) * 128],
                                h2a[:, k * BC + g * BW : k * BC + g * BW + BW],
                                start=(k == 0),
                                stop=(k == NK3 - 1),
                            )
                        if NPE and g == NBC - 1:
                            flush_pe(upto=a - 1)
                        at = apool.tile([128, BW], ST)
                        if i3 % 4 != 3:
                            nc.scalar.activation(
                                at[:, :], ps3[:, :], Relu,
                                bias=b3t[:, a : a + 1],
                            )
                        else:
                            nc.vector.tensor_scalar(
                                at[:, :], ps3[:, :], b3t[:, a : a + 1], 0.0,
                                op0=add, op1=mx,
                            )
                        if a in pe_set:
                            pend.append((a, g, at))
                            continue
                        w4s = w4t[:, a : a + 1]
                        stt_cnt[g] += 1
                        if stt_cnt[g] % 2 == 1:
                            if seedv[g]:
                                seedv[g] = False
                                nc.vector.tensor_scalar(
                                    accv[g][:, :], at[:, :], w4s, b4t[:, 0:1],
                                    op0=mult, op1=add,
                                )
                            else:
                                nc.vector.scalar_tensor_tensor(
                                    accv[g][:, :], at[:, :], w4s,
                                    accv[g][:, :], op0=mult, op1=add,
                                )
                        else:
                            if seedg[g]:
                                seedg[g] = False
                                nc.vector.tensor_scalar(
                                    accg[g][:, :], at[:, :], w4s, 0.0,
                                    op0=mult, op1=add,
                                )
                            else:
                                nc.vector.scalar_tensor_tensor(
                                    accg[g][:, :], at[:, :], w4s,
                                    accg[g][:, :], op0=mult, op1=add,
                                )
                        if a == last_stt:
                            # pre-merge the two stt chains while the PE is
                            # still streaming the remaining diag matmuls
                            nc.vector.tensor_add(
                                outsb[:, g * BW : (g + 1) * BW],
                                accv[g][:, :], accg[g][:, :],
                            )
                if NPE:
                    flush_pe()
                for g in range(NBC):
                    if NPE:
                        nc.vector.tensor_add(
                            outsb[:, g * BW : (g + 1) * BW],
                            outsb[:, g * BW : (g + 1) * BW],
                            po[g][:, :],
                        )
                    nc.sync.dma_start(
                        out_d[:, g * BW : (g + 1) * BW],
                        outsb[:, g * BW : (g + 1) * BW],
                    )
            else:
                # ---- L3+L4: aT chunks then block-diag W4 reduction ----
                po = [
                    ps.tile([128, BW], FP, name=f"po{i}", tag=f"pst{4 + i}")
                    for i in range(NBC)
                ]
                rot = ["pst6", "pst7", "pst2", "pst0"]
                w3ab = [w3a, w3b]
                pend = []

                def flush_l4(upto=None):
                    keep = []
                    for (pmi, pb, pw4, pat) in pend:
                        if upto is not None and pmi > upto:
                            keep.append((pmi, pb, pw4, pat))
                            continue
                        nc.tensor.matmul(
                            po[pb][:, :], pw4, pat[:, :],
                            start=(pmi == 0), stop=(pmi == NM3 - 1),
                        )
                    pend[:] = keep

                for mi in range(NM3):
                    for b in range(NBC):
                        i3 = mi * NBC + b
                        ps3 = ps.tile(
                            [128, BW], FP, name=f"ps3_{i3}", tag=rot[i3 % 4]
                        )
                        for k in range(NK3):
                            nc.tensor.matmul(
                                ps3[:, :],
                                w3ab[k][:, mi * 128 : (mi + 1) * 128],
                                h2a[:, k * BC + b * BW : k * BC + b * BW + BW],
                                start=(k == 0),
                                stop=(k == NK3 - 1),
                            )
                        if b == NBC - 1:
                            flush_l4(upto=mi - 1)
                        at = apool.tile([128, BW], ST)
                        if i3 % 5 < 3:
                            nc.scalar.activation(
                                at[:, :], ps3[:, :], Relu, bias=b3t[:, mi : mi + 1]
                            )
                        else:
                            nc.vector.tensor_scalar(
                                at[:, :], ps3[:, :], b3t[:, mi : mi + 1], 0.0,
                                op0=mybir.AluOpType.add, op1=mybir.AluOpType.max,
                            )
                        pend.append(
                            (mi, b, w4a[:, mi * 128 : (mi + 1) * 128], at)
                        )
                        if mi == NM3 - 1:
                            flush_l4()
                nc.scalar.activation(
                    outsb[:, 0:BW], po[0][:, :], Ident, bias=b4t[:, 0:1]
                )
                nc.sync.dma_start(out_d[:, 0:BW], outsb[:, 0:BW])
                nc.vector.tensor_scalar_add(
                    outsb[:, BW:BC], po[1][:, :], b4t[:, 0:1]
                )
                nc.sync.dma_start(out_d[:, BW:BC], outsb[:, BW:BC])

    nc.compile()
    return nc


def _get_nc(mode: str):
    if mode not in _CACHE:
        if mode.startswith("v9") or mode.startswith("v10"):
            _CACHE[mode] = _build_v9(mode)
        elif mode.startswith("v"):
            _CACHE[mode] = _build_v3(mode)
        else:
            _CACHE[mode] = _build(mode)
    return _CACHE[mode]


def _interleave_k(mat_gp: np.ndarray) -> np.ndarray:
    """[GP, F] -> [128, NKP*F] with col block ki = k-tile ki."""
    f = mat_gp.shape[1]
    return np.ascontiguousarray(
        mat_gp.reshape(NKP, 128, f).transpose(1, 0, 2).reshape(128, NKP * f)
    )


def _prep_inputs(x, W1, b1, W2, b2, W3, b3, W4, b4, mode="f32r"):
    f = np.float32
    if mode.startswith("bf16") or mode.startswith("v"):
        import ml_dtypes

        st = np.dtype(ml_dtypes.bfloat16)
    else:
        st = np.dtype(np.float32)
    ac = np.ascontiguousarray

    def cst(a):
        return a if a.dtype == st else a.astype(st)

    V9 = mode.startswith("v9") or mode.startswith("v10")
    V10 = mode.startswith("v10")
    if V9:
        import ml_dtypes

        f8 = np.dtype(ml_dtypes.float8_e4m3)
        NP8 = int(mode.split("_")[1]) if "_" in mode else 20
        NKF8 = 2 * NP8
        K8 = NKF8 * 128
        SX, SW, SB = 2.0 ** -1, 2.0 ** 6, 2.0 ** 5
    else:
        SX = SW = SB = 1.0
        K8 = NKF8 = 0

    x = np.asarray(x, f)
    xTp = np.zeros((GP, B), st)
    np.copyto(xTp[:G], cst(x.T * SX))                          # [GP, B]
    w1tp = np.zeros((GP, H1), st)
    np.copyto(w1tp[:G], cst(np.asarray(W1, f).T * SW))
    if V9:
        # fp8 region: k-tiles 0..NKF8-1 quantized e4m3 from the scaled f32
        xT8 = ac((x.T[:K8] * SX).astype(f8))                   # [K8, B]
        w18p = ac((np.asarray(W1, f).T[:K8] * SW).astype(f8))  # [K8, H1]
        w18i = ac(
            w18p.reshape(NKF8, 128, H1).transpose(1, 0, 2).reshape(128, NKF8 * H1)
        )
        xTp = xTp[K8:]                                         # bf16 region
        w1tp = w1tp[K8:]

        def _ik(mat):  # [nk*128, F] -> [128, nk*F]
            nk = mat.shape[0] // 128
            fdim = mat.shape[1]
            return np.ascontiguousarray(
                mat.reshape(nk, 128, fdim).transpose(1, 0, 2).reshape(128, nk * fdim)
            )

        w1ti = _ik(w1tp)
    else:
        w1ti = _interleave_k(w1tp)                             # [128, NKP*H1]
    b1r = ac(np.asarray(b1, f).reshape(NM1, 128).T * SB)       # [128, 4]
    w2T = cst(np.asarray(W2, f).T / SB)                        # [H1, H2]
    if mode.startswith("v"):
        # k-interleaved single-DMA layout: w2i[p, ki*H2+c] = W2T[ki*128+p, c]
        w2t = ac(w2T.reshape(NK2, 128, H2).transpose(1, 0, 2).reshape(128, NK2 * H2))
        w2key = "w2i"
    else:
        w2t = ac(w2T)
        w2key = "w2t"
    b2r = ac(np.asarray(b2, f).reshape(NM2, 128).T)            # [128, 2]
    if V10:
        # w3f[h, a*P + p] = W3[p, h, a]; b3/w4 land per-partition over p
        w3f = ac(cst(np.asarray(W3, f).transpose(1, 2, 0).reshape(H2, A * P)))
        b3r = ac(np.asarray(b3, f))                                # [128, 64]
        w4r = ac(np.asarray(W4, f))                                # [128, 64]
        b4r = ac(np.asarray(b4, f).reshape(128, 1))
        shared = {
            "w1ti": w1ti, "b1r": b1r, w2key: w2t, "b2r": b2r,
            "w3f": w3f, "b3r": b3r, "w4r": w4r, "b4r": b4r,
        }
        NPE = int(mode.split("_")[2]) if mode.count("_") > 1 else 30
        if NPE:
            # diag(W4[:, a_j]) for the PE-reduced a's (even spread over A)
            pe_as = [
                a for a in range(A) if (a + 1) * NPE // A > a * NPE // A
            ]
            w4d = np.zeros((128, NPE * 128), np.float32)
            for j, aj in enumerate(pe_as):
                w4d[np.arange(128), j * 128 + np.arange(128)] = np.asarray(
                    W4, f
                )[:, aj]
            shared["w4d"] = ac(cst(w4d))
    else:
        w3f = ac(cst(np.asarray(W3, f).transpose(1, 0, 2).reshape(H2, PA)))
        b3r = ac(np.asarray(b3, f).reshape(PA).reshape(NM3, 128).T)  # [128, 64]
        w4bd = np.zeros((PA, P), st)
        w4bd[np.arange(PA), np.arange(PA) // A] = cst(np.asarray(W4, f).reshape(PA))
        # k-tile-interleaved block-diag W4: w4i[p, mi*128+c] = w4bd[mi*128+p, c]
        w4i = ac(w4bd.reshape(NM3, 128, P).transpose(1, 0, 2).reshape(128, NM3 * P))
        b4r = ac(np.asarray(b4, f).reshape(128, 1))
        shared = {
            "w1ti": w1ti, "b1r": b1r, w2key: w2t, "b2r": b2r,
            "w3f": w3f, "b3r": b3r, "w4i": w4i, "b4r": b4r,
        }
    if V9:
        shared["w18i"] = w18i
    in_maps = []
    for c in range(NCORES):
        if V9:
            m = {
                "xTi": _ik(xTp[:, c * BC : (c + 1) * BC]),
                "x8i": _ik(xT8[:, c * BC : (c + 1) * BC]),
            }
        else:
            m = {"xTi": _interleave_k(xTp[:, c * BC : (c + 1) * BC])}
        m.update(shared)
        in_maps.append(m)
    return in_maps


def run_with_results(inputs: dict, trace: bool = False, mode: str | None = None):
    """Returns (full_output [B, P] float32, BassKernelResults)."""
    from concourse.bass_utils import run_bass_kernel_spmd

    if mode is None:
        mode = os.environ.get("CTP_MODE", "v10")
    nc = _get_nc(mode)
    in_maps = _prep_inputs(**inputs, mode=mode)
    res = run_bass_kernel_spmd(
        nc, in_maps, core_ids=list(range(NCORES)), trace=trace
    )
    out = np.empty((B, P), np.float32)
    for c in range(NCORES):
        out[c * BC : (c + 1) * BC, :] = res.results[c]["out"].T
    return out, res


def kernel(**inputs) -> np.ndarray:
    out, _ = run_with_results(inputs, trace=False)
    return out

